# revision 35
# baseline (speedup 1.0000x reference)
"""DeepseekV4-style attention (partial-RoPE LoRA-Q GQA sliding-window) on 8
Trainium2 NeuronCores.

Sharding: core c = 4*b + g handles batch b (of 2) and GQA group g (of 4):
q heads 4g..4g+3, kv head g, the matching column slices of Wq_eff/Wk/Wv and
row slice of Wo.  Each core computes a partial output; the host sums the four
partials per batch.

Design notes:
- LoRA Q projection folded on the host (W_eff = Wqa @ Wqb slice).
- Host packs hidden/weights into the exact SBUF layouts so each tensor needs
  a handful of large DMAs; startup is finely interleaved so the first matmul
  fires ~2us in.
- RoPE's half-swap is a signed 64x64 permutation matmul on the PE; rope and
  V-transpose emissions for block b are deferred into block b+1's k-loop so
  the PE FIFO is never blocked by a DVE-dependent instruction.
- Attention tiles narrowed at causal/window edges (N even, >=256 to satisfy
  s3d3 fp32r ISA rules at full rate); chunk loop is software-pipelined
  (QK of chunk c+2 is emitted before PV/pd of chunk c).
- Output projection for q-block qb-1 is emitted interleaved after each head
  of q-block qb, so stage-4 matmuls fill the PE while DVE finishes the
  softmax normalization of qb.
"""

import numpy as np
import ml_dtypes
import concourse.bass as bass
import concourse.mybir as mybir
import concourse.tile as tile
from concourse.bass_utils import run_bass_kernel_spmd

F32 = mybir.dt.float32
F32R = mybir.dt.float32r
BF16 = mybir.dt.bfloat16
ACTF = mybir.ActivationFunctionType
ALU = mybir.AluOpType

B, S, D = 2, 2048, 2048
H, KVH, HD = 16, 4, 128
ROT, LORA, WINDOW = 64, 512, 1024
ROPE_BASE = 10000.0
SCALE = HD ** -0.5

HPC = H // KVH          # 4 q heads per core
SB = 512                # free-dim block for matmuls
NSB = S // SB           # 4 seq blocks
KT = D // 128           # 16 contraction tiles over D
ST = S // 128           # 16 seq 128-chunks
N_CORES = 8
PIPE = 1                # attention chunk software-pipeline depth


def _split_multiwaits(nc):
    """This image's walrus accepts only one embedded SyncWait per instruction;
    split Tile's multi-wait sync_infos into standalone event-semaphore waits."""
    n = 0
    for func in nc.m.functions:
        for bb in func.blocks:
            insts = list(bb.instructions)
            out = []
            changed = False
            for inst in insts:
                si = inst.sync_info
                if si is not None and si.on_wait and len(si.on_wait) > 1:
                    waits = list(si.on_wait)
                    for w in waits[:-1]:
                        ev = mybir.InstEventSemaphore(
                            name=f"{inst.name}_wsplit_{n}", ins=[], outs=[]
                        )
                        ev.engine = inst.engine
                        ev.sync_info = mybir.SyncInfo(on_wait=[w], on_update=[])
                        out.append(ev)
                        n += 1
                    inst.sync_info = mybir.SyncInfo(
                        on_wait=[waits[-1]], on_update=list(si.on_update or [])
                    )
                    changed = True
                out.append(inst)
            if changed:
                bb.instructions = out
    return n


def build_nc():
    nc = bass.Bass()
    # host-packed layouts: hid col = blk*8192 + k*512 + c; weff col = k*512+c;
    # wkv col = k*256+c; wo col = h*2048+c
    hid = nc.dram_tensor("hid", [128, NSB * KT * SB], BF16, kind="ExternalInput")
    weff = nc.dram_tensor("weff", [128, KT * 512], BF16, kind="ExternalInput")
    wkv = nc.dram_tensor("wkv", [128, KT * 256], BF16, kind="ExternalInput")
    wo = nc.dram_tensor("wo", [128, HPC * D], BF16, kind="ExternalInput")
    rcs = nc.dram_tensor("rcs", [128, S], BF16, kind="ExternalInput")
    out = nc.dram_tensor("out", [S, D], BF16, kind="ExternalOutput")

    with tile.TileContext(nc) as tc:
        with (
            tc.tile_pool(name="cst", bufs=1) as cst,
            tc.tile_pool(name="big", bufs=1) as big,
        ):
            # ---- small constants (engine-built, no DMA) ----
            onesf = cst.tile([128, 128], F32, tag="onesf")
            nc.vector.memset(onesf[:], 1.0)
            ones = cst.tile([128, 128], BF16, tag="ones")
            nc.vector.tensor_copy(ones[:], onesf[:])
            identf = cst.tile([128, 128], F32, tag="identf")
            nc.gpsimd.affine_select(
                out=identf[:], in_=onesf[:], pattern=[[1, 128]],
                compare_op=ALU.is_equal, fill=0.0, base=0, channel_multiplier=-1,
            )
            ident = cst.tile([128, 128], BF16, tag="ident")
            nc.vector.tensor_copy(ident[:], identf[:])
            # signed rope permutation P: P[32+i, i] = -1, P[j, 32+j] = +1
            negf = cst.tile([64, 32], F32, tag="negf")
            nc.vector.memset(negf[:], -1.0)
            posf = cst.tile([64, 32], F32, tag="posf")
            nc.vector.memset(posf[:], 1.0)
            permf = cst.tile([64, 64], F32, tag="permf")
            nc.gpsimd.affine_select(
                out=permf[:, 0:32], in_=negf[:], pattern=[[-1, 32]],
                compare_op=ALU.is_equal, fill=0.0, base=-32, channel_multiplier=1,
            )
            nc.gpsimd.affine_select(
                out=permf[:, 32:64], in_=posf[:], pattern=[[-1, 32]],
                compare_op=ALU.is_equal, fill=0.0, base=0, channel_multiplier=1,
            )
            perm = cst.tile([64, 64], BF16, tag="perm")
            nc.vector.tensor_copy(perm[:], permf[:])
            warm = cst.tile([128, 512], BF16, tag="warm")
            nc.vector.memset(warm[:], 0.0)

            # ---- persistent activations, split per seq-block so a late
            # rope/transpose on block b never false-blocks attention reads of
            # earlier blocks (dep tracking is tile-granular) ----
            qTs = [big.tile([128, HPC * SB], BF16, tag=f"qT{b}", name=f"qT{b}")
                   for b in range(NSB)]          # col = h*SB + c
            kTs = [big.tile([128, SB], BF16, tag=f"kT{b}", name=f"kT{b}")
                   for b in range(NSB)]
            vnats = [big.tile([128, SB], BF16, tag=f"vn{b}", name=f"vn{b}")
                     for b in range(NSB)]        # V rows, chunk t at cols (t%4)*128
            wo_sb = big.tile([128, HPC * D], BF16, tag="wo_sb")

            with (
                tc.tile_pool(name="wp", bufs=1) as wp,
                tc.tile_pool(name="hp", bufs=5) as hp,
                tc.tile_pool(name="vt", bufs=2) as vt,
                tc.tile_pool(name="rp", bufs=2) as rp,
            ):
              with (
                tc.tile_pool(name="psA", bufs=1, space="PSUM") as psA,
                tc.tile_pool(name="psT", bufs=1, space="PSUM") as psT,
                tc.tile_pool(name="psP", bufs=1, space="PSUM") as psP,
              ):
                  weff_sb = wp.tile([128, KT * 512], BF16, tag="weff_sb")
                  wkv_sb = wp.tile([128, KT * 256], BF16, tag="wkv_sb")
                  # bf16 cos/sin: all-bf16 rope operands put the DVE in its
                  # 2x 16-bit mode, halving rope cost
                  ropeCC = wp.tile([64, S], BF16, tag="ropeCC")
                  ropeSS = wp.tile([64, S], BF16, tag="ropeSS")

                  # PE warmup: dependency-free matmuls ramp the PE p-state and
                  # keep it busy while the first weight/hidden DMAs land.
                  for _ in range(18):
                      pPw = psP.tile([64, SB], F32, tag="pP")
                      nc.tensor.matmul(pPw[:], perm[:], warm[0:64, :],
                                       start=True, stop=True)

                  def dma_w(g0, g1):
                      # weights ride the Activation HWDGE queue so they don't
                      # serialize behind hidden-state loads on the SP queue
                      nc.scalar.dma_start(
                          out=weff_sb[:, g0 * 512:g1 * 512],
                          in_=weff[:, g0 * 512:g1 * 512],
                      )
                      nc.scalar.dma_start(
                          out=wkv_sb[:, g0 * 256:g1 * 256],
                          in_=wkv[:, g0 * 256:g1 * 256],
                      )

                  def dma_ht(sb_i, qd):
                      t = hp.tile([128, 2048], BF16, tag="ht", name=f"ht_{sb_i}_{qd}")
                      nc.sync.dma_start(
                          out=t[:],
                          in_=hid[:, sb_i * 8192 + qd * 2048:
                                  sb_i * 8192 + (qd + 1) * 2048],
                      )
                      return t

                  # startup interleave: weight k-groups racing block-0 hidden
                  h0t = []
                  dma_w(0, 1)
                  t00 = hp.tile([128, 2048], BF16, tag="ht", name="ht_0_0")
                  nc.sync.dma_start(out=t00[:, 0:512], in_=hid[:, 0:512])
                  nc.sync.dma_start(out=t00[:, 512:1024], in_=hid[:, 512:1024])
                  h0t.append(t00)
                  dma_w(1, 4)
                  nc.sync.dma_start(out=t00[:, 1024:2048], in_=hid[:, 1024:2048])
                  h0t.append(dma_ht(0, 1))
                  dma_w(4, 8)
                  h0t.append(dma_ht(0, 2))
                  dma_w(8, 12)
                  h0t.append(dma_ht(0, 3))
                  dma_w(12, 16)
                  nc.scalar.dma_start(out=ropeCC[:], in_=rcs[0:64, :])
                  nc.scalar.dma_start(out=ropeSS[:], in_=rcs[64:128, :])

                  def rope_emit_dma(dst, csl, rsl):
                      # PE-free variant for the last block's flush: the swap
                      # goes through two small SBUF->SBUF DMAs instead of the
                      # permutation matmul, keeping the PE FIFO clear at the
                      # stage-1 -> stage-3 transition.
                      swp = rp.tile([64, SB], BF16, tag="swp")
                      nc.sync.dma_start(out=swp[0:32, :], in_=dst[32:64, csl])
                      nc.sync.dma_start(out=swp[32:64, :], in_=dst[0:32, csl])
                      csb = rp.tile([64, SB], BF16, tag="csb2")
                      nc.vector.tensor_mul(csb[:], dst[0:64, csl], ropeCC[:, rsl])
                      tsin = rp.tile([64, SB], BF16, tag="tsin2")
                      nc.vector.tensor_mul(tsin[:], swp[:], ropeSS[:, rsl])
                      nc.vector.tensor_sub(dst[0:32, csl], csb[0:32, :], tsin[0:32, :])
                      nc.vector.tensor_add(dst[32:64, csl], csb[32:64, :], tsin[32:64, :])

                  def rope_emit(dst, csl, rsl):
                      # dst rows 0:64 hold [x1; x2]; out = x*cos + P^T(x*sin)
                      tsin = rp.tile([64, SB], BF16, tag="tsin")
                      nc.vector.tensor_mul(tsin[:], dst[0:64, csl], ropeSS[:, rsl])
                      csb = rp.tile([64, SB], BF16, tag="csb")
                      nc.vector.tensor_mul(csb[:], dst[0:64, csl], ropeCC[:, rsl])
                      pP = psP.tile([64, SB], F32, tag="pP")
                      nc.tensor.matmul(pP[:], perm[:], tsin[:], start=True, stop=True)
                      nc.vector.tensor_add(dst[0:64, csl], csb[:], pP[:])

                  # ---- stage 1: q^T (folded), k^T, v -> vnat ----
                  pending = []   # deferred rope/transpose emissions from prev block
                  for sb_i in range(NSB):
                      sl = slice(sb_i * SB, (sb_i + 1) * SB)
                      if sb_i > 0:
                          hts = [dma_ht(sb_i, qd) for qd in range(4)]
                      else:
                          hts = h0t
                      pq = [
                          psA.tile([128, SB], F32, tag=f"pq{m}", name=f"pq{m}_{sb_i}")
                          for m in range(HPC)
                      ]
                      pk = psA.tile([128, SB], F32, tag="pk")
                      pv = psA.tile([128, SB], F32, tag="pv")
                      for k in range(KT):
                          hsl = hts[k // 4][:, (k % 4) * 512:(k % 4 + 1) * 512]
                          st, sp = (k == 0), (k == KT - 1)
                          for m in range(HPC):
                              nc.tensor.matmul(
                                  pq[m][:],
                                  weff_sb[:, k * 512 + m * 128: k * 512 + (m + 1) * 128],
                                  hsl, start=st, stop=sp,
                              )
                          nc.tensor.matmul(
                              pk[:], wkv_sb[:, k * 256: k * 256 + 128], hsl,
                              start=st, stop=sp,
                          )
                          nc.tensor.matmul(
                              pv[:], wkv_sb[:, k * 256 + 128: k * 256 + 256], hsl,
                              start=st, stop=sp,
                          )
                          if k in (2, 4, 6, 8, 10, 12) and pending:
                              pending.pop(0)()
                      # evacuate this block; defer rope/transposes into next block
                      newpend = []
                      last = sb_i == NSB - 1
                      vtmp = vt.tile([128, SB], BF16, tag="vtmp")
                      nc.scalar.copy(vtmp[:], pv[:])

                      def emit_transposes(sb_i=sb_i, vtmp=vtmp):
                          # bf16 V chunks transpose through the DMA XBAR —
                          # no PE/DVE/PSUM involvement at all.  Scalar queue:
                          # the SP queue carries hidden loads + outputs.
                          for tt in range(4):
                              nc.scalar.dma_start_transpose(
                                  out=vnats[sb_i][:, tt * 128:(tt + 1) * 128],
                                  in_=vtmp[:, tt * 128:(tt + 1) * 128],
                              )
                      fn = rope_emit_dma if last else rope_emit
                      for m in range(HPC):
                          qsl = slice(m * SB, (m + 1) * SB)
                          # alternate evacuation copies between ACT and DVE so
                          # the attention phase (whose PSUM pools reuse these
                          # banks) isn't gated on one serial ACT chain
                          if m % 2 == 0:
                              nc.scalar.copy(qTs[sb_i][:, qsl], pq[m][:])
                          else:
                              nc.vector.tensor_copy(qTs[sb_i][:, qsl], pq[m][:])
                          newpend.append(
                              lambda qsl=qsl, sl=sl, fn=fn, b=sb_i:
                              fn(qTs[b], qsl, sl)
                          )
                      nc.vector.tensor_copy(kTs[sb_i][:], pk[:])
                      newpend.append(
                          lambda sl=sl, fn=fn, b=sb_i: fn(kTs[b], slice(0, SB), sl)
                      )
                      if last:
                          # final flush order: transposes (PE) before the
                          # PE-free DMA-ropes so the PE is not left waiting
                          # behind the DVE rope chains at the stage boundary
                          newpend.insert(0, emit_transposes)
                      else:
                          newpend.append(emit_transposes)
                      for fn in pending:   # anything not yet flushed
                          fn()
                      pending = newpend
                  # transposes for the last block flush now; its 5 DMA-ropes
                  # are spread into qb0's head loop below (SBUF pools stay open)
                  pending[0]()
                  pending = pending[1:]

              # ---- Wo prefetch (lands well before stage 4 needs it) ----
              nc.scalar.dma_start(out=wo_sb[:], in_=wo[:, :])

              # ---- stage 3+4: attention with interleaved output projection ----
              with (
                  tc.tile_pool(name="at", bufs=2) as at,
                  tc.tile_pool(name="ex", bufs=5) as ex,
                  tc.tile_pool(name="rc", bufs=2) as rc,
                  tc.tile_pool(name="ob", bufs=2) as ob,
                  tc.tile_pool(name="psL", bufs=2, space="PSUM") as psL,
                  tc.tile_pool(name="psO", bufs=2, space="PSUM") as psO,
                  tc.tile_pool(name="psD", bufs=2, space="PSUM") as psD,
                  tc.tile_pool(name="psW", bufs=2, space="PSUM") as psW,
              ):
                  def stage4_chunk(qbx, tl, atile):
                      # out[q-chunk t, :] partial = attn(:, t-cols) @ Wo
                      t = qbx * 4 + tl
                      obuf = ob.tile([128, D], BF16, tag="obuf")
                      for n4 in range(4):
                          pw = psW.tile([128, SB], F32, tag="pw")
                          for hh in range(HPC):
                              nc.tensor.matmul(
                                  pw[:],
                                  atile[:, hh * SB + tl * 128:
                                        hh * SB + (tl + 1) * 128],
                                  wo_sb[:, hh * D + n4 * SB: hh * D + (n4 + 1) * SB],
                                  start=(hh == 0), stop=(hh == HPC - 1),
                              )
                          o0 = n4 * SB
                          # split PSUM→SBUF copies between ACT and DVE: ACT
                          # also carries every exp, DVE the normalization
                          if n4 % 2 == 0:
                              nc.vector.tensor_copy(obuf[:, o0:o0 + SB], pw[:])
                          else:
                              nc.scalar.copy(obuf[:, o0:o0 + SB], pw[:])
                          if n4 == 1:
                              nc.sync.dma_start(
                                  out=out[t * 128:(t + 1) * 128, 0:2 * SB],
                                  in_=obuf[:, 0:2 * SB],
                              )
                          elif n4 == 3:
                              nc.sync.dma_start(
                                  out=out[t * 128:(t + 1) * 128, 2 * SB:D],
                                  in_=obuf[:, 2 * SB:D],
                              )

                  prev_attnT = None
                  prev_qb = None
                  for qi, qb in enumerate([1, 2, 3, 0]):
                      q0 = qb * SB
                      kt_lo = max(0, q0 - WINDOW + 1) // 128
                      kt_hi = q0 // 128 + 3
                      attnT = at.tile([128, HPC * SB], BF16, tag="attnT",
                                      name=f"attnT_{qb}")
                      for h in range(HPC):
                          po = psO.tile([128, SB], F32, tag="po")
                          pd = psD.tile([128, SB], F32, tag="pd")
                          inflight = []

                          def flush_one():
                              kt, lo, hi, e = inflight.pop(0)
                              kb, kc = kt // 4, (kt % 4) * 128
                              st, sp = (kt == kt_lo), (kt == kt_hi)
                              nc.tensor.matmul(
                                  po[:, lo:hi], vnats[kb][:, kc:kc + 128],
                                  e[:, lo:hi], start=st, stop=sp,
                              )
                              nc.tensor.matmul(
                                  pd[:, lo:hi], ones[:], e[:, lo:hi],
                                  start=st, stop=sp,
                              )

                          for kt in range(kt_lo, kt_hi + 1):
                              dp = kt * 128 - q0
                              # exact valid cols [lo, hi): bf16 matmuls have no
                              # fp32r moving-dim ISA constraint, so no padding
                              lo = max(0, dp)
                              hi = min(SB, dp + WINDOW + 128)
                              kb, kc = kt // 4, (kt % 4) * 128
                              pl = psL.tile([128, SB], F32, tag="pl")
                              nc.tensor.matmul(
                                  pl[:, lo:hi], kTs[kb][:, kc:kc + 128],
                                  qTs[qb][:, h * SB + lo: h * SB + hi],
                                  start=True, stop=True,
                              )
                              e = ex.tile([128, SB], BF16, tag="e")
                              nc.scalar.activation(
                                  e[:, lo:hi], pl[:, lo:hi], ACTF.Exp, scale=SCALE
                              )
                              # the mask staircase spans at most 128 cols (one
                              # per key partition) — select only on that band
                              if dp >= 0:
                                  # causal: keep f - dp - j >= 0 on [lo, lo+n2)
                                  n2 = min(128, hi - lo)
                                  nc.gpsimd.affine_select(
                                      out=e[:, lo:lo + n2], in_=e[:, lo:lo + n2],
                                      pattern=[[1, n2]], compare_op=ALU.is_ge,
                                      fill=0.0, base=lo - dp, channel_multiplier=-1,
                                  )
                              elif dp <= -(WINDOW - SB + 1):
                                  # window: keep W-1 + dp + j - f >= 0 on the
                                  # last <=128 cols [l2, hi)
                                  l2 = max(lo, hi - 128)
                                  nc.gpsimd.affine_select(
                                      out=e[:, l2:hi], in_=e[:, l2:hi],
                                      pattern=[[-1, hi - l2]], compare_op=ALU.is_ge,
                                      fill=0.0, base=WINDOW - 1 + dp - l2,
                                      channel_multiplier=1,
                                  )
                              inflight.append((kt, lo, hi, e))
                              if len(inflight) > PIPE:
                                  flush_one()
                          while inflight:
                              flush_one()
                          # evacuate po to SBUF right away so its PSUM bank
                          # frees without waiting for the slow reciprocal —
                          # PE's PV matmuls two heads later reuse that bank
                          poS = rc.tile([128, SB], BF16, tag="poS")
                          nc.vector.tensor_copy(poS[:], po[:])
                          rec = rc.tile([128, SB], F32, tag="rec")
                          nc.vector.reciprocal(rec[:], pd[:])
                          nc.vector.tensor_mul(
                              attnT[:, h * SB:(h + 1) * SB], poS[:], rec[:]
                          )
                          if prev_attnT is not None:
                              stage4_chunk(prev_qb, h, prev_attnT)
                          # rope pops wait until qb1 is done: its heads have no
                          # stage-4 fill, so the DVE must not be loaded there
                          if qi >= 1 and pending:
                              pending.pop(0)()
                              if qi == 1 and h == 0 and pending:
                                  pending.pop(0)()
                      prev_attnT = attnT
                      prev_qb = qb
                  for tl in range(4):
                      stage4_chunk(prev_qb, tl, prev_attnT)
    _split_multiwaits(nc)
    return nc


_NC = None


def _get_nc():
    global _NC
    if _NC is None:
        _NC = build_nc()
    return _NC


def _make_in_maps(hidden, position_ids, Wqa, Wqb, Wk, Wv, Wo):
    hidden = np.asarray(hidden, dtype=np.float32)
    position_ids = np.asarray(position_ids)
    Wqa = np.asarray(Wqa, dtype=np.float32)
    Wqb = np.asarray(Wqb, dtype=np.float32)
    Wk = np.asarray(Wk, dtype=np.float32)
    Wv = np.asarray(Wv, dtype=np.float32)
    Wo = np.asarray(Wo, dtype=np.float32)
    weff_full = Wqa @ Wqb  # [D, H*HD]; exact assoc. fold of the LoRA Q proj

    inv_freq = 1.0 / (ROPE_BASE ** (np.arange(0, ROT, 2, dtype=np.float32) / ROT))
    in_maps = []
    for c in range(N_CORES):
        b, g = c // KVH, c % KVH
        pos = position_ids[b].astype(np.float32)
        freqs = pos[:, None] * inv_freq[None, :]        # [S, 32]
        cosT = np.cos(freqs).T.astype(np.float32)       # [32, S]
        sinT = np.sin(freqs).T.astype(np.float32)
        rcs = np.concatenate([cosT, cosT, sinT, sinT], axis=0).astype(
            ml_dtypes.bfloat16)  # [128, S]
        hsb = (hidden[b].T.reshape(KT, 128, NSB, SB)
               .transpose(1, 2, 0, 3).reshape(128, NSB * KT * SB))
        weff = (weff_full[:, g * HPC * HD:(g + 1) * HPC * HD]
                .reshape(KT, 128, 512).transpose(1, 0, 2).reshape(128, KT * 512))
        wkv = np.concatenate(
            [Wk[:, g * HD:(g + 1) * HD], Wv[:, g * HD:(g + 1) * HD]], axis=1
        ).reshape(KT, 128, 256).transpose(1, 0, 2).reshape(128, KT * 256)
        wog = (Wo[g * HPC * HD:(g + 1) * HPC * HD, :]
               .reshape(HPC, 128, D).transpose(1, 0, 2).reshape(128, HPC * D))
        in_maps.append({
            "hid": np.ascontiguousarray(hsb.astype(ml_dtypes.bfloat16)),
            "weff": np.ascontiguousarray(weff.astype(ml_dtypes.bfloat16)),
            "wkv": np.ascontiguousarray(wkv.astype(ml_dtypes.bfloat16)),
            "wo": np.ascontiguousarray(wog.astype(ml_dtypes.bfloat16)),
            "rcs": np.ascontiguousarray(rcs),
        })
    return in_maps


def _run(inputs, trace=False):
    nc = _get_nc()
    in_maps = _make_in_maps(**inputs)
    res = run_bass_kernel_spmd(nc, in_maps, list(range(N_CORES)), trace=trace)
    out = np.zeros((B, S, D), dtype=np.float32)
    for c in range(N_CORES):
        out[c // KVH] += res.results[c]["out"].astype(np.float32)
    return out, res


def kernel(**inputs) -> np.ndarray:
    return _run(inputs, trace=False)[0]



# revision 38
# speedup vs baseline: 1.0318x; 1.0318x over previous
"""DeepseekV4-style attention (partial-RoPE LoRA-Q GQA sliding-window) on 8
Trainium2 NeuronCores.

Sharding: core c = 4*b + g handles batch b (of 2) and GQA group g (of 4):
q heads 4g..4g+3, kv head g, the matching column slices of Wq_eff/Wk/Wv and
row slice of Wo.  Each core computes a partial output; the host sums the four
partials per batch.

Design notes:
- LoRA Q projection folded on the host (W_eff = Wqa @ Wqb slice).
- Host packs hidden/weights into the exact SBUF layouts so each tensor needs
  a handful of large DMAs; startup is finely interleaved so the first matmul
  fires ~2us in.
- RoPE's half-swap is a signed 64x64 permutation matmul on the PE; rope and
  V-transpose emissions for block b are deferred into block b+1's k-loop so
  the PE FIFO is never blocked by a DVE-dependent instruction.
- Attention tiles narrowed at causal/window edges (N even, >=256 to satisfy
  s3d3 fp32r ISA rules at full rate); chunk loop is software-pipelined
  (QK of chunk c+2 is emitted before PV/pd of chunk c).
- Output projection for q-block qb-1 is emitted interleaved after each head
  of q-block qb, so stage-4 matmuls fill the PE while DVE finishes the
  softmax normalization of qb.
"""

import numpy as np
import ml_dtypes
import concourse.bass as bass
import concourse.mybir as mybir
import concourse.tile as tile
from concourse.bass_utils import run_bass_kernel_spmd

F32 = mybir.dt.float32
F32R = mybir.dt.float32r
BF16 = mybir.dt.bfloat16
ACTF = mybir.ActivationFunctionType
ALU = mybir.AluOpType

B, S, D = 2, 2048, 2048
H, KVH, HD = 16, 4, 128
ROT, LORA, WINDOW = 64, 512, 1024
ROPE_BASE = 10000.0
SCALE = HD ** -0.5

HPC = H // KVH          # 4 q heads per core
SB = 512                # free-dim block for matmuls
NSB = S // SB           # 4 seq blocks
KT = D // 128           # 16 contraction tiles over D
ST = S // 128           # 16 seq 128-chunks
N_CORES = 8
PIPE = 1                # attention chunk software-pipeline depth


def _split_multiwaits(nc):
    """This image's walrus accepts only one embedded SyncWait per instruction;
    split Tile's multi-wait sync_infos into standalone event-semaphore waits."""
    n = 0
    for func in nc.m.functions:
        for bb in func.blocks:
            insts = list(bb.instructions)
            out = []
            changed = False
            for inst in insts:
                si = inst.sync_info
                if si is not None and si.on_wait and len(si.on_wait) > 1:
                    waits = list(si.on_wait)
                    for w in waits[:-1]:
                        ev = mybir.InstEventSemaphore(
                            name=f"{inst.name}_wsplit_{n}", ins=[], outs=[]
                        )
                        ev.engine = inst.engine
                        ev.sync_info = mybir.SyncInfo(on_wait=[w], on_update=[])
                        out.append(ev)
                        n += 1
                    inst.sync_info = mybir.SyncInfo(
                        on_wait=[waits[-1]], on_update=list(si.on_update or [])
                    )
                    changed = True
                out.append(inst)
            if changed:
                bb.instructions = out
    return n


def build_nc():
    nc = bass.Bass()
    # host-packed layouts: hid col = blk*8192 + k*512 + c; weff col = k*512+c;
    # wkv col = k*256+c; wo col = h*2048+c
    hid = nc.dram_tensor("hid", [128, NSB * KT * SB], BF16, kind="ExternalInput")
    weff = nc.dram_tensor("weff", [128, KT * 512], BF16, kind="ExternalInput")
    wkv = nc.dram_tensor("wkv", [128, KT * 256], BF16, kind="ExternalInput")
    wo = nc.dram_tensor("wo", [128, HPC * D], BF16, kind="ExternalInput")
    rcs = nc.dram_tensor("rcs", [128, S], BF16, kind="ExternalInput")
    out = nc.dram_tensor("out", [S, D], BF16, kind="ExternalOutput")

    with tile.TileContext(nc) as tc:
        with (
            tc.tile_pool(name="cst", bufs=1) as cst,
            tc.tile_pool(name="big", bufs=1) as big,
        ):
            # ---- small constants (engine-built, no DMA) ----
            onesf = cst.tile([128, 128], F32, tag="onesf")
            nc.vector.memset(onesf[:], 1.0)
            ones = cst.tile([128, 128], BF16, tag="ones")
            nc.vector.tensor_copy(ones[:], onesf[:])
            identf = cst.tile([128, 128], F32, tag="identf")
            nc.gpsimd.affine_select(
                out=identf[:], in_=onesf[:], pattern=[[1, 128]],
                compare_op=ALU.is_equal, fill=0.0, base=0, channel_multiplier=-1,
            )
            ident = cst.tile([128, 128], BF16, tag="ident")
            nc.vector.tensor_copy(ident[:], identf[:])
            # signed rope permutation P: P[32+i, i] = -1, P[j, 32+j] = +1
            negf = cst.tile([64, 32], F32, tag="negf")
            nc.vector.memset(negf[:], -1.0)
            posf = cst.tile([64, 32], F32, tag="posf")
            nc.vector.memset(posf[:], 1.0)
            permf = cst.tile([64, 64], F32, tag="permf")
            nc.gpsimd.affine_select(
                out=permf[:, 0:32], in_=negf[:], pattern=[[-1, 32]],
                compare_op=ALU.is_equal, fill=0.0, base=-32, channel_multiplier=1,
            )
            nc.gpsimd.affine_select(
                out=permf[:, 32:64], in_=posf[:], pattern=[[-1, 32]],
                compare_op=ALU.is_equal, fill=0.0, base=0, channel_multiplier=1,
            )
            perm = cst.tile([64, 64], BF16, tag="perm")
            nc.vector.tensor_copy(perm[:], permf[:])
            warm = cst.tile([128, 512], BF16, tag="warm")
            nc.vector.memset(warm[:], 0.0)

            # ---- persistent activations, split per seq-block so a late
            # rope/transpose on block b never false-blocks attention reads of
            # earlier blocks (dep tracking is tile-granular) ----
            qTs = [big.tile([128, HPC * SB], BF16, tag=f"qT{b}", name=f"qT{b}")
                   for b in range(NSB)]          # col = h*SB + c
            kTs = [big.tile([128, SB], BF16, tag=f"kT{b}", name=f"kT{b}")
                   for b in range(NSB)]
            vnats = [big.tile([128, SB], BF16, tag=f"vn{b}", name=f"vn{b}")
                     for b in range(NSB)]        # V rows, chunk t at cols (t%4)*128
            wo_sb = big.tile([128, HPC * D], BF16, tag="wo_sb")

            with (
                tc.tile_pool(name="wp", bufs=1) as wp,
                tc.tile_pool(name="hp", bufs=5) as hp,
                tc.tile_pool(name="vt", bufs=2) as vt,
                tc.tile_pool(name="rp", bufs=2) as rp,
            ):
              with (
                tc.tile_pool(name="psA", bufs=1, space="PSUM") as psA,
                tc.tile_pool(name="psT", bufs=1, space="PSUM") as psT,
                tc.tile_pool(name="psP", bufs=1, space="PSUM") as psP,
              ):
                  weff_sb = wp.tile([128, KT * 512], BF16, tag="weff_sb")
                  wkv_sb = wp.tile([128, KT * 256], BF16, tag="wkv_sb")
                  # bf16 cos/sin: all-bf16 rope operands put the DVE in its
                  # 2x 16-bit mode, halving rope cost
                  ropeCC = wp.tile([64, S], BF16, tag="ropeCC")
                  ropeSS = wp.tile([64, S], BF16, tag="ropeSS")

                  # PE warmup: dependency-free matmuls ramp the PE p-state and
                  # keep it busy while the first weight/hidden DMAs land.
                  for _ in range(18):
                      pPw = psP.tile([64, SB], F32, tag="pP")
                      nc.tensor.matmul(pPw[:], perm[:], warm[0:64, :],
                                       start=True, stop=True)

                  def dma_w(g0, g1):
                      # weights ride the Activation HWDGE queue so they don't
                      # serialize behind hidden-state loads on the SP queue
                      nc.scalar.dma_start(
                          out=weff_sb[:, g0 * 512:g1 * 512],
                          in_=weff[:, g0 * 512:g1 * 512],
                      )
                      nc.scalar.dma_start(
                          out=wkv_sb[:, g0 * 256:g1 * 256],
                          in_=wkv[:, g0 * 256:g1 * 256],
                      )

                  def dma_ht(sb_i, qd):
                      t = hp.tile([128, 2048], BF16, tag="ht", name=f"ht_{sb_i}_{qd}")
                      nc.sync.dma_start(
                          out=t[:],
                          in_=hid[:, sb_i * 8192 + qd * 2048:
                                  sb_i * 8192 + (qd + 1) * 2048],
                      )
                      return t

                  # startup interleave: weight k-groups racing block-0 hidden
                  h0t = []
                  dma_w(0, 1)
                  t00 = hp.tile([128, 2048], BF16, tag="ht", name="ht_0_0")
                  nc.sync.dma_start(out=t00[:, 0:512], in_=hid[:, 0:512])
                  nc.sync.dma_start(out=t00[:, 512:1024], in_=hid[:, 512:1024])
                  h0t.append(t00)
                  dma_w(1, 4)
                  nc.sync.dma_start(out=t00[:, 1024:2048], in_=hid[:, 1024:2048])
                  h0t.append(dma_ht(0, 1))
                  dma_w(4, 8)
                  h0t.append(dma_ht(0, 2))
                  dma_w(8, 12)
                  h0t.append(dma_ht(0, 3))
                  dma_w(12, 16)
                  nc.scalar.dma_start(out=ropeCC[:], in_=rcs[0:64, :])
                  nc.scalar.dma_start(out=ropeSS[:], in_=rcs[64:128, :])

                  def rope_emit_dma(dst, csl, rsl):
                      # PE-free variant for the last block's flush: the swap
                      # goes through two small SBUF->SBUF DMAs instead of the
                      # permutation matmul, keeping the PE FIFO clear at the
                      # stage-1 -> stage-3 transition.
                      swp = rp.tile([64, SB], BF16, tag="swp")
                      nc.sync.dma_start(out=swp[0:32, :], in_=dst[32:64, csl])
                      nc.sync.dma_start(out=swp[32:64, :], in_=dst[0:32, csl])
                      csb = rp.tile([64, SB], BF16, tag="csb2")
                      nc.vector.tensor_mul(csb[:], dst[0:64, csl], ropeCC[:, rsl])
                      tsin = rp.tile([64, SB], BF16, tag="tsin2")
                      nc.vector.tensor_mul(tsin[:], swp[:], ropeSS[:, rsl])
                      nc.vector.tensor_sub(dst[0:32, csl], csb[0:32, :], tsin[0:32, :])
                      nc.vector.tensor_add(dst[32:64, csl], csb[32:64, :], tsin[32:64, :])

                  def rope_emit(dst, csl, rsl):
                      # dst rows 0:64 hold [x1; x2]; out = x*cos + P^T(x*sin)
                      tsin = rp.tile([64, SB], BF16, tag="tsin")
                      nc.vector.tensor_mul(tsin[:], dst[0:64, csl], ropeSS[:, rsl])
                      csb = rp.tile([64, SB], BF16, tag="csb")
                      nc.vector.tensor_mul(csb[:], dst[0:64, csl], ropeCC[:, rsl])
                      pP = psP.tile([64, SB], F32, tag="pP")
                      nc.tensor.matmul(pP[:], perm[:], tsin[:], start=True, stop=True)
                      nc.vector.tensor_add(dst[0:64, csl], csb[:], pP[:])

                  # ---- stage 1: q^T (folded), k^T, v -> vnat ----
                  pending = []   # deferred rope/transpose emissions from prev block
                  for sb_i in range(NSB):
                      sl = slice(sb_i * SB, (sb_i + 1) * SB)
                      if sb_i > 0:
                          hts = [dma_ht(sb_i, qd) for qd in range(4)]
                      else:
                          hts = h0t
                      pq = [
                          psA.tile([128, SB], F32, tag=f"pq{m}", name=f"pq{m}_{sb_i}")
                          for m in range(HPC)
                      ]
                      pk = psA.tile([128, SB], F32, tag="pk")
                      pv = psA.tile([128, SB], F32, tag="pv")
                      for k in range(KT):
                          hsl = hts[k // 4][:, (k % 4) * 512:(k % 4 + 1) * 512]
                          st, sp = (k == 0), (k == KT - 1)
                          for m in range(HPC):
                              nc.tensor.matmul(
                                  pq[m][:],
                                  weff_sb[:, k * 512 + m * 128: k * 512 + (m + 1) * 128],
                                  hsl, start=st, stop=sp,
                              )
                          nc.tensor.matmul(
                              pk[:], wkv_sb[:, k * 256: k * 256 + 128], hsl,
                              start=st, stop=sp,
                          )
                          nc.tensor.matmul(
                              pv[:], wkv_sb[:, k * 256 + 128: k * 256 + 256], hsl,
                              start=st, stop=sp,
                          )
                          if k in (2, 4, 6, 8, 10, 12) and pending:
                              pending.pop(0)()
                      # evacuate this block; defer rope/transposes into next block
                      newpend = []
                      last = sb_i == NSB - 1
                      vtmp = vt.tile([128, SB], BF16, tag="vtmp")
                      nc.scalar.copy(vtmp[:], pv[:])

                      def emit_transposes(sb_i=sb_i, vtmp=vtmp):
                          # bf16 V chunks transpose through the DMA XBAR —
                          # no PE/DVE/PSUM involvement at all.  SP queue: on
                          # the ACT queue these block the in-order ACT engine
                          # right when attention exps start.
                          for tt in range(4):
                              nc.sync.dma_start_transpose(
                                  out=vnats[sb_i][:, tt * 128:(tt + 1) * 128],
                                  in_=vtmp[:, tt * 128:(tt + 1) * 128],
                              )
                      fn = rope_emit_dma if last else rope_emit
                      for m in range(HPC):
                          qsl = slice(m * SB, (m + 1) * SB)
                          # alternate evacuation copies between ACT and DVE so
                          # the attention phase (whose PSUM pools reuse these
                          # banks) isn't gated on one serial ACT chain
                          if m % 2 == 0:
                              nc.scalar.copy(qTs[sb_i][:, qsl], pq[m][:])
                          else:
                              nc.vector.tensor_copy(qTs[sb_i][:, qsl], pq[m][:])
                          newpend.append(
                              lambda qsl=qsl, sl=sl, fn=fn, b=sb_i:
                              fn(qTs[b], qsl, sl)
                          )
                      nc.vector.tensor_copy(kTs[sb_i][:], pk[:])
                      newpend.append(
                          lambda sl=sl, fn=fn, b=sb_i: fn(kTs[b], slice(0, SB), sl)
                      )
                      if last:
                          # final flush order: transposes (PE) before the
                          # PE-free DMA-ropes so the PE is not left waiting
                          # behind the DVE rope chains at the stage boundary
                          newpend.insert(0, emit_transposes)
                      else:
                          newpend.append(emit_transposes)
                      for fn in pending:   # anything not yet flushed
                          fn()
                      pending = newpend
                  # transposes for the last block flush now; its 5 DMA-ropes
                  # are spread into qb0's head loop below (SBUF pools stay open)
                  pending[0]()
                  pending = pending[1:]

              # ---- Wo prefetch (lands well before stage 4 needs it) ----
              nc.scalar.dma_start(out=wo_sb[:], in_=wo[:, :])

              # ---- stage 3+4: attention with interleaved output projection ----
              with (
                  tc.tile_pool(name="at", bufs=2) as at,
                  tc.tile_pool(name="ex", bufs=5) as ex,
                  tc.tile_pool(name="rc", bufs=2) as rc,
                  tc.tile_pool(name="ob", bufs=2) as ob,
                  tc.tile_pool(name="psL", bufs=2, space="PSUM") as psL,
                  tc.tile_pool(name="psO", bufs=2, space="PSUM") as psO,
                  tc.tile_pool(name="psD", bufs=2, space="PSUM") as psD,
                  tc.tile_pool(name="psW", bufs=2, space="PSUM") as psW,
              ):
                  def stage4_chunk(qbx, tl, atile, dve_copies=True):
                      # out[q-chunk t, :] partial = attn(:, t-cols) @ Wo
                      t = qbx * 4 + tl
                      obuf = ob.tile([128, D], BF16, tag="obuf")
                      for n4 in range(4):
                          pw = psW.tile([128, SB], F32, tag="pw")
                          for hh in range(HPC):
                              nc.tensor.matmul(
                                  pw[:],
                                  atile[:, hh * SB + tl * 128:
                                        hh * SB + (tl + 1) * 128],
                                  wo_sb[:, hh * D + n4 * SB: hh * D + (n4 + 1) * SB],
                                  start=(hh == 0), stop=(hh == HPC - 1),
                              )
                          o0 = n4 * SB
                          # split PSUM→SBUF copies between ACT and DVE: ACT
                          # also carries every exp, DVE the normalization
                          if dve_copies and n4 % 2 == 0:
                              nc.vector.tensor_copy(obuf[:, o0:o0 + SB], pw[:])
                          else:
                              nc.scalar.copy(obuf[:, o0:o0 + SB], pw[:])
                          if n4 == 1:
                              nc.sync.dma_start(
                                  out=out[t * 128:(t + 1) * 128, 0:2 * SB],
                                  in_=obuf[:, 0:2 * SB],
                              )
                          elif n4 == 3:
                              nc.sync.dma_start(
                                  out=out[t * 128:(t + 1) * 128, 2 * SB:D],
                                  in_=obuf[:, 2 * SB:D],
                              )

                  prev_attnT = None
                  prev_qb = None
                  for qi, qb in enumerate([1, 2, 3, 0]):
                      q0 = qb * SB
                      kt_lo = max(0, q0 - WINDOW + 1) // 128
                      kt_hi = q0 // 128 + 3
                      attnT = at.tile([128, HPC * SB], BF16, tag="attnT",
                                      name=f"attnT_{qb}")
                      for h in range(HPC):
                          po = psO.tile([128, SB], F32, tag="po")
                          pd = psD.tile([128, SB], F32, tag="pd")
                          inflight = []

                          def flush_one():
                              kt, lo, hi, e = inflight.pop(0)
                              kb, kc = kt // 4, (kt % 4) * 128
                              st, sp = (kt == kt_lo), (kt == kt_hi)
                              nc.tensor.matmul(
                                  po[:, lo:hi], vnats[kb][:, kc:kc + 128],
                                  e[:, lo:hi], start=st, stop=sp,
                              )
                              nc.tensor.matmul(
                                  pd[:, lo:hi], ones[:], e[:, lo:hi],
                                  start=st, stop=sp,
                              )

                          for kt in range(kt_lo, kt_hi + 1):
                              dp = kt * 128 - q0
                              # exact valid cols [lo, hi): bf16 matmuls have no
                              # fp32r moving-dim ISA constraint, so no padding
                              lo = max(0, dp)
                              hi = min(SB, dp + WINDOW + 128)
                              kb, kc = kt // 4, (kt % 4) * 128
                              pl = psL.tile([128, SB], F32, tag="pl")
                              nc.tensor.matmul(
                                  pl[:, lo:hi], kTs[kb][:, kc:kc + 128],
                                  qTs[qb][:, h * SB + lo: h * SB + hi],
                                  start=True, stop=True,
                              )
                              e = ex.tile([128, SB], BF16, tag="e")
                              nc.scalar.activation(
                                  e[:, lo:hi], pl[:, lo:hi], ACTF.Exp, scale=SCALE
                              )
                              # the mask staircase spans at most 128 cols (one
                              # per key partition) — select only on that band
                              if dp >= 0:
                                  # causal: keep f - dp - j >= 0 on [lo, lo+n2)
                                  n2 = min(128, hi - lo)
                                  nc.gpsimd.affine_select(
                                      out=e[:, lo:lo + n2], in_=e[:, lo:lo + n2],
                                      pattern=[[1, n2]], compare_op=ALU.is_ge,
                                      fill=0.0, base=lo - dp, channel_multiplier=-1,
                                  )
                              elif dp <= -(WINDOW - SB + 1):
                                  # window: keep W-1 + dp + j - f >= 0 on the
                                  # last <=128 cols [l2, hi)
                                  l2 = max(lo, hi - 128)
                                  nc.gpsimd.affine_select(
                                      out=e[:, l2:hi], in_=e[:, l2:hi],
                                      pattern=[[-1, hi - l2]], compare_op=ALU.is_ge,
                                      fill=0.0, base=WINDOW - 1 + dp - l2,
                                      channel_multiplier=1,
                                  )
                              inflight.append((kt, lo, hi, e))
                              if len(inflight) > PIPE:
                                  flush_one()
                          while inflight:
                              flush_one()
                          # evacuate po to SBUF right away so its PSUM bank
                          # frees without waiting for the slow reciprocal —
                          # PE's PV matmuls two heads later reuse that bank
                          poS = rc.tile([128, SB], BF16, tag="poS")
                          nc.vector.tensor_copy(poS[:], po[:])
                          rec = rc.tile([128, SB], F32, tag="rec")
                          nc.vector.reciprocal(rec[:], pd[:])
                          nc.vector.tensor_mul(
                              attnT[:, h * SB:(h + 1) * SB], poS[:], rec[:]
                          )
                          if prev_attnT is not None:
                              # during the last (cheapest) qb, DVE carries the
                              # whole normalization chain — keep obuf copies
                              # off it there
                              stage4_chunk(prev_qb, h, prev_attnT,
                                           dve_copies=(qi < 3))
                          # rope pops wait until qb1 is done: its heads have no
                          # stage-4 fill, so the DVE must not be loaded there
                          if qi >= 1 and pending:
                              pending.pop(0)()
                              if qi == 1 and h == 0 and pending:
                                  pending.pop(0)()
                      prev_attnT = attnT
                      prev_qb = qb
                  for tl in range(4):
                      stage4_chunk(prev_qb, tl, prev_attnT)
    _split_multiwaits(nc)
    return nc


_NC = None


def _get_nc():
    global _NC
    if _NC is None:
        _NC = build_nc()
    return _NC


def _make_in_maps(hidden, position_ids, Wqa, Wqb, Wk, Wv, Wo):
    hidden = np.asarray(hidden, dtype=np.float32)
    position_ids = np.asarray(position_ids)
    Wqa = np.asarray(Wqa, dtype=np.float32)
    Wqb = np.asarray(Wqb, dtype=np.float32)
    Wk = np.asarray(Wk, dtype=np.float32)
    Wv = np.asarray(Wv, dtype=np.float32)
    Wo = np.asarray(Wo, dtype=np.float32)
    weff_full = Wqa @ Wqb  # [D, H*HD]; exact assoc. fold of the LoRA Q proj

    inv_freq = 1.0 / (ROPE_BASE ** (np.arange(0, ROT, 2, dtype=np.float32) / ROT))
    in_maps = []
    for c in range(N_CORES):
        b, g = c // KVH, c % KVH
        pos = position_ids[b].astype(np.float32)
        freqs = pos[:, None] * inv_freq[None, :]        # [S, 32]
        cosT = np.cos(freqs).T.astype(np.float32)       # [32, S]
        sinT = np.sin(freqs).T.astype(np.float32)
        rcs = np.concatenate([cosT, cosT, sinT, sinT], axis=0).astype(
            ml_dtypes.bfloat16)  # [128, S]
        hsb = (hidden[b].T.reshape(KT, 128, NSB, SB)
               .transpose(1, 2, 0, 3).reshape(128, NSB * KT * SB))
        weff = (weff_full[:, g * HPC * HD:(g + 1) * HPC * HD]
                .reshape(KT, 128, 512).transpose(1, 0, 2).reshape(128, KT * 512))
        wkv = np.concatenate(
            [Wk[:, g * HD:(g + 1) * HD], Wv[:, g * HD:(g + 1) * HD]], axis=1
        ).reshape(KT, 128, 256).transpose(1, 0, 2).reshape(128, KT * 256)
        wog = (Wo[g * HPC * HD:(g + 1) * HPC * HD, :]
               .reshape(HPC, 128, D).transpose(1, 0, 2).reshape(128, HPC * D))
        in_maps.append({
            "hid": np.ascontiguousarray(hsb.astype(ml_dtypes.bfloat16)),
            "weff": np.ascontiguousarray(weff.astype(ml_dtypes.bfloat16)),
            "wkv": np.ascontiguousarray(wkv.astype(ml_dtypes.bfloat16)),
            "wo": np.ascontiguousarray(wog.astype(ml_dtypes.bfloat16)),
            "rcs": np.ascontiguousarray(rcs),
        })
    return in_maps


def _run(inputs, trace=False):
    nc = _get_nc()
    in_maps = _make_in_maps(**inputs)
    res = run_bass_kernel_spmd(nc, in_maps, list(range(N_CORES)), trace=trace)
    out = np.zeros((B, S, D), dtype=np.float32)
    for c in range(N_CORES):
        out[c // KVH] += res.results[c]["out"].astype(np.float32)
    return out, res


def kernel(**inputs) -> np.ndarray:
    return _run(inputs, trace=False)[0]



# revision 40
# speedup vs baseline: 1.0621x; 1.0294x over previous
"""DeepseekV4-style attention (partial-RoPE LoRA-Q GQA sliding-window) on 8
Trainium2 NeuronCores.

Sharding: core c = 4*b + g handles batch b (of 2) and GQA group g (of 4):
q heads 4g..4g+3, kv head g, the matching column slices of Wq_eff/Wk/Wv and
row slice of Wo.  Each core computes a partial output; the host sums the four
partials per batch (in f32; the device ships bf16 partials).

Design notes:
- LoRA Q projection folded on the host (W_eff = Wqa @ Wqb slice).
- Everything downstream of the PSUM accumulators is bf16: qT/kT/vnat/e/attnT.
  bf16 matmuls run 1 cycle/row at any moving-dim size (no fp32r >=256 rule),
  so attention tiles are trimmed exactly; rope runs in the DVE 2x 16-bit mode.
- Stage 1 runs each seq block in TWO passes of 3 PSUM accumulators rotating
  over 4 banks, which leaves 3 PSUM banks free during stage 1.  Those banks
  host a bufs=1 psL/psO/psD set used to compute qb0's attention interleaved
  into blocks 2-3's k-loops: the first attention block rides under stage-1
  matmul backpressure instead of idling the PE on its exp/recip latencies.
- Masking: causal/window staircases span at most 128 cols (one per key
  partition), so affine_select only touches that band.
- V chunks transpose through the DMA XBAR (bf16), not the PE.
- The per-head softmax normalization evacuates po to SBUF immediately (PSUM
  bank frees without waiting on the slow DVE reciprocal).
- Main attention (qb1..qb3) interleaves stage-4 output projection of the
  previous qb after each head, so the PE never drains on DVE tails; the
  final flush projects qb3.
"""

import numpy as np
import ml_dtypes
import concourse.bass as bass
import concourse.mybir as mybir
import concourse.tile as tile
from concourse.bass_utils import run_bass_kernel_spmd

F32 = mybir.dt.float32
F32R = mybir.dt.float32r
BF16 = mybir.dt.bfloat16
ACTF = mybir.ActivationFunctionType
ALU = mybir.AluOpType

B, S, D = 2, 2048, 2048
H, KVH, HD = 16, 4, 128
ROT, LORA, WINDOW = 64, 512, 1024
ROPE_BASE = 10000.0
SCALE = HD ** -0.5

HPC = H // KVH          # 4 q heads per core
SB = 512                # free-dim block for matmuls
NSB = S // SB           # 4 seq blocks
KT = D // 128           # 16 contraction tiles over D
ST = S // 128           # 16 seq 128-chunks
N_CORES = 8
PIPE = 1                # attention chunk software-pipeline depth
SKEW = 2                # stage-1 pass k-skew (late accumulators)


def _split_multiwaits(nc):
    """This image's walrus accepts only one embedded SyncWait per instruction;
    split Tile's multi-wait sync_infos into standalone event-semaphore waits."""
    n = 0
    for func in nc.m.functions:
        for bb in func.blocks:
            insts = list(bb.instructions)
            out = []
            changed = False
            for inst in insts:
                si = inst.sync_info
                if si is not None and si.on_wait and len(si.on_wait) > 1:
                    waits = list(si.on_wait)
                    for w in waits[:-1]:
                        ev = mybir.InstEventSemaphore(
                            name=f"{inst.name}_wsplit_{n}", ins=[], outs=[]
                        )
                        ev.engine = inst.engine
                        ev.sync_info = mybir.SyncInfo(on_wait=[w], on_update=[])
                        out.append(ev)
                        n += 1
                    inst.sync_info = mybir.SyncInfo(
                        on_wait=[waits[-1]], on_update=list(si.on_update or [])
                    )
                    changed = True
                out.append(inst)
            if changed:
                bb.instructions = out
    return n


def build_nc():
    nc = bass.Bass()
    # host-packed layouts: hid col = blk*8192 + k*512 + c; weff col = k*512+c;
    # wkv col = k*256+c; wo col = h*2048+c
    hid = nc.dram_tensor("hid", [128, NSB * KT * SB], BF16, kind="ExternalInput")
    weff = nc.dram_tensor("weff", [128, KT * 512], BF16, kind="ExternalInput")
    wkv = nc.dram_tensor("wkv", [128, KT * 256], BF16, kind="ExternalInput")
    wo = nc.dram_tensor("wo", [128, HPC * D], BF16, kind="ExternalInput")
    rcs = nc.dram_tensor("rcs", [128, S], BF16, kind="ExternalInput")
    out = nc.dram_tensor("out", [S, D], BF16, kind="ExternalOutput")

    with tile.TileContext(nc) as tc:
        with (
            tc.tile_pool(name="cst", bufs=1) as cst,
            tc.tile_pool(name="big", bufs=1) as big,
        ):
            # ---- small constants (engine-built, no DMA) ----
            onesf = cst.tile([128, 128], F32, tag="onesf")
            nc.vector.memset(onesf[:], 1.0)
            ones = cst.tile([128, 128], BF16, tag="ones")
            nc.vector.tensor_copy(ones[:], onesf[:])
            # signed rope permutation P: P[32+i, i] = -1, P[j, 32+j] = +1
            negf = cst.tile([64, 32], F32, tag="negf")
            nc.vector.memset(negf[:], -1.0)
            posf = cst.tile([64, 32], F32, tag="posf")
            nc.vector.memset(posf[:], 1.0)
            permf = cst.tile([64, 64], F32, tag="permf")
            nc.gpsimd.affine_select(
                out=permf[:, 0:32], in_=negf[:], pattern=[[-1, 32]],
                compare_op=ALU.is_equal, fill=0.0, base=-32, channel_multiplier=1,
            )
            nc.gpsimd.affine_select(
                out=permf[:, 32:64], in_=posf[:], pattern=[[-1, 32]],
                compare_op=ALU.is_equal, fill=0.0, base=0, channel_multiplier=1,
            )
            perm = cst.tile([64, 64], BF16, tag="perm")
            nc.vector.tensor_copy(perm[:], permf[:])
            warm = cst.tile([128, 512], BF16, tag="warm")
            nc.vector.memset(warm[:], 0.0)

            # ---- persistent activations, split per seq-block so a late
            # rope/transpose on block b never false-blocks attention reads of
            # earlier blocks (dep tracking is tile-granular) ----
            qTs = [big.tile([128, HPC * SB], BF16, tag=f"qT{b}", name=f"qT{b}")
                   for b in range(NSB)]          # col = h*SB + c
            kTs = [big.tile([128, SB], BF16, tag=f"kT{b}", name=f"kT{b}")
                   for b in range(NSB)]
            vnats = [big.tile([128, SB], BF16, tag=f"vn{b}", name=f"vn{b}")
                     for b in range(NSB)]        # V rows, chunk t at cols (t%4)*128
            wo_sb = big.tile([128, HPC * D], BF16, tag="wo_sb")

            with (
                tc.tile_pool(name="wp", bufs=1) as wp,
                tc.tile_pool(name="hp", bufs=5) as hp,
                tc.tile_pool(name="vt", bufs=2) as vt,
                tc.tile_pool(name="rp", bufs=2) as rp,
                tc.tile_pool(name="at", bufs=2) as at,
                tc.tile_pool(name="ex", bufs=5) as ex,
                tc.tile_pool(name="rc", bufs=2) as rc,
                tc.tile_pool(name="ob", bufs=2) as ob,
            ):
              weff_sb = wp.tile([128, KT * 512], BF16, tag="weff_sb")
              wkv_sb = wp.tile([128, KT * 256], BF16, tag="wkv_sb")
              # bf16 cos/sin: all-bf16 rope operands put the DVE in its
              # 2x 16-bit mode, halving rope cost
              ropeCC = wp.tile([64, S], BF16, tag="ropeCC")
              ropeSS = wp.tile([64, S], BF16, tag="ropeSS")

              def rope_emit_dma(dst, csl, rsl):
                  # PE-free variant for the last block's flush: the half-swap
                  # goes through two small SBUF->SBUF DMAs instead of the
                  # permutation matmul, keeping the PE FIFO clear inside the
                  # main attention loop.
                  swp = rp.tile([64, SB], BF16, tag="swp")
                  nc.sync.dma_start(out=swp[0:32, :], in_=dst[32:64, csl])
                  nc.sync.dma_start(out=swp[32:64, :], in_=dst[0:32, csl])
                  csb = rp.tile([64, SB], BF16, tag="csb2")
                  nc.vector.tensor_mul(csb[:], dst[0:64, csl], ropeCC[:, rsl])
                  tsin = rp.tile([64, SB], BF16, tag="tsin2")
                  nc.vector.tensor_mul(tsin[:], swp[:], ropeSS[:, rsl])
                  nc.vector.tensor_sub(dst[0:32, csl], csb[0:32, :], tsin[0:32, :])
                  nc.vector.tensor_add(dst[32:64, csl], csb[32:64, :], tsin[32:64, :])

              def make_attn_gen(qb, psLp, psOp, psDp, attnT):
                  # generator emitting one qb's attention; yields
                  # ("chunk", h) after each chunk and ("head", h) after each
                  # head's normalization tail, so the caller paces emission
                  def gen():
                      q0 = qb * SB
                      kt_lo = max(0, q0 - WINDOW + 1) // 128
                      kt_hi = q0 // 128 + 3
                      for h in range(HPC):
                          po = psOp.tile([128, SB], F32, tag="po")
                          pd = psDp.tile([128, SB], F32, tag="pd")
                          inflight = []

                          def flush_one(po=po, pd=pd, inflight=inflight,
                                        kt_lo=kt_lo, kt_hi=kt_hi):
                              kt, lo, hi, e = inflight.pop(0)
                              kb, kc = kt // 4, (kt % 4) * 128
                              st, sp = (kt == kt_lo), (kt == kt_hi)
                              nc.tensor.matmul(
                                  po[:, lo:hi], vnats[kb][:, kc:kc + 128],
                                  e[:, lo:hi], start=st, stop=sp,
                              )
                              nc.tensor.matmul(
                                  pd[:, lo:hi], ones[:], e[:, lo:hi],
                                  start=st, stop=sp,
                              )

                          for kt in range(kt_lo, kt_hi + 1):
                              dp = kt * 128 - q0
                              # exact valid cols [lo, hi): bf16 matmuls have no
                              # fp32r moving-dim constraint, so no padding
                              lo = max(0, dp)
                              hi = min(SB, dp + WINDOW + 128)
                              kb, kc = kt // 4, (kt % 4) * 128
                              pl = psLp.tile([128, SB], F32, tag="pl")
                              nc.tensor.matmul(
                                  pl[:, lo:hi], kTs[kb][:, kc:kc + 128],
                                  qTs[qb][:, h * SB + lo: h * SB + hi],
                                  start=True, stop=True,
                              )
                              e = ex.tile([128, SB], BF16, tag="e")
                              nc.scalar.activation(
                                  e[:, lo:hi], pl[:, lo:hi], ACTF.Exp, scale=SCALE
                              )
                              # the mask staircase spans at most 128 cols (one
                              # per key partition) — select only on that band
                              if dp >= 0:
                                  n2 = min(128, hi - lo)
                                  nc.gpsimd.affine_select(
                                      out=e[:, lo:lo + n2], in_=e[:, lo:lo + n2],
                                      pattern=[[1, n2]], compare_op=ALU.is_ge,
                                      fill=0.0, base=lo - dp,
                                      channel_multiplier=-1,
                                  )
                              elif dp <= -(WINDOW - SB + 1):
                                  l2 = max(lo, hi - 128)
                                  nc.gpsimd.affine_select(
                                      out=e[:, l2:hi], in_=e[:, l2:hi],
                                      pattern=[[-1, hi - l2]],
                                      compare_op=ALU.is_ge,
                                      fill=0.0, base=WINDOW - 1 + dp - l2,
                                      channel_multiplier=1,
                                  )
                              inflight.append((kt, lo, hi, e))
                              if len(inflight) > PIPE:
                                  flush_one()
                              yield ("chunk", h)
                          while inflight:
                              flush_one()
                          # evacuate po to SBUF right away so its PSUM bank
                          # frees without waiting for the slow reciprocal
                          poS = rc.tile([128, SB], BF16, tag="poS")
                          nc.vector.tensor_copy(poS[:], po[:])
                          rec = rc.tile([128, SB], F32, tag="rec")
                          nc.vector.reciprocal(rec[:], pd[:])
                          nc.vector.tensor_mul(
                              attnT[:, h * SB:(h + 1) * SB], poS[:], rec[:]
                          )
                          yield ("head", h)
                  return gen()

              # ================= stage 1 (+ overlapped qb0) =================
              with (
                  tc.tile_pool(name="psA", bufs=1, space="PSUM") as psA,
                  tc.tile_pool(name="psP", bufs=1, space="PSUM") as psP,
                  tc.tile_pool(name="psL0", bufs=1, space="PSUM") as psL0,
                  tc.tile_pool(name="psO0", bufs=1, space="PSUM") as psO0,
                  tc.tile_pool(name="psD0", bufs=1, space="PSUM") as psD0,
              ):
                  # PE warmup: dependency-free matmuls ramp the PE p-state and
                  # keep it busy while the first weight/hidden DMAs land.
                  for _ in range(18):
                      pPw = psP.tile([64, SB], F32, tag="pP")
                      nc.tensor.matmul(pPw[:], perm[:], warm[0:64, :],
                                       start=True, stop=True)

                  def dma_w(g0, g1):
                      # weights ride the Activation HWDGE queue so they don't
                      # serialize behind hidden-state loads on the SP queue
                      nc.scalar.dma_start(
                          out=weff_sb[:, g0 * 512:g1 * 512],
                          in_=weff[:, g0 * 512:g1 * 512],
                      )
                      nc.scalar.dma_start(
                          out=wkv_sb[:, g0 * 256:g1 * 256],
                          in_=wkv[:, g0 * 256:g1 * 256],
                      )

                  def dma_ht(sb_i, qd):
                      t = hp.tile([128, 2048], BF16, tag="ht",
                                  name=f"ht_{sb_i}_{qd}")
                      nc.sync.dma_start(
                          out=t[:],
                          in_=hid[:, sb_i * 8192 + qd * 2048:
                                  sb_i * 8192 + (qd + 1) * 2048],
                      )
                      return t

                  # startup interleave: weight k-groups racing block-0 hidden
                  h0t = []
                  dma_w(0, 1)
                  t00 = hp.tile([128, 2048], BF16, tag="ht", name="ht_0_0")
                  nc.sync.dma_start(out=t00[:, 0:512], in_=hid[:, 0:512])
                  nc.sync.dma_start(out=t00[:, 512:1024], in_=hid[:, 512:1024])
                  h0t.append(t00)
                  dma_w(1, 4)
                  nc.sync.dma_start(out=t00[:, 1024:2048], in_=hid[:, 1024:2048])
                  h0t.append(dma_ht(0, 1))
                  dma_w(4, 8)
                  h0t.append(dma_ht(0, 2))
                  dma_w(8, 12)
                  h0t.append(dma_ht(0, 3))
                  dma_w(12, 16)
                  nc.scalar.dma_start(out=ropeCC[:], in_=rcs[0:64, :])
                  nc.scalar.dma_start(out=ropeSS[:], in_=rcs[64:128, :])
                  # Wo prefetch: scalar queue is idle once weights are in;
                  # wo is not needed until the first stage-4 (~100us later)
                  nc.scalar.dma_start(out=wo_sb[:], in_=wo[:, :])

                  def rope_emit(dst, csl, rsl):
                      # dst rows 0:64 hold [x1; x2]; out = x*cos + P^T(x*sin)
                      tsin = rp.tile([64, SB], BF16, tag="tsin")
                      nc.vector.tensor_mul(tsin[:], dst[0:64, csl], ropeSS[:, rsl])
                      csb = rp.tile([64, SB], BF16, tag="csb")
                      nc.vector.tensor_mul(csb[:], dst[0:64, csl], ropeCC[:, rsl])
                      pP = psP.tile([64, SB], F32, tag="pP")
                      nc.tensor.matmul(pP[:], perm[:], tsin[:], start=True,
                                       stop=True)
                      nc.vector.tensor_add(dst[0:64, csl], csb[:], pP[:])

                  # qb0's attention, interleaved into blocks 2-3 below
                  attnT0 = at.tile([128, HPC * SB], BF16, tag="attnT",
                                   name="attnT_q0")
                  genq0 = make_attn_gen(0, psL0, psO0, psD0, attnT0)
                  genq0_done = [False]

                  def gen_step():
                      if not genq0_done[0] and next(genq0, None) is None:
                          genq0_done[0] = True

                  # 4 rotating PSUM accumulators; each pass takes 3, frees 3.
                  # Taking from the left reuses the longest-evacuated banks.
                  ring = [psA.tile([128, SB], F32, tag=f"acc{i}",
                                   name=f"acc{i}") for i in range(4)]

                  pending = []   # prev block's deferred rope/transpose pops
                  for sb_i in range(NSB):
                      sl = slice(sb_i * SB, (sb_i + 1) * SB)
                      if sb_i > 0:
                          hts = [dma_ht(sb_i, qd) for qd in range(4)]
                      else:
                          hts = h0t
                      newpend = []
                      last = sb_i == NSB - 1
                      fn = rope_emit_dma if last else rope_emit
                      for pas in range(2):
                          a0, a1, a2 = ring[0], ring[1], ring[2]
                          ring = ring[3:] + [a0, a1, a2]
                          # pass A: q heads 0,1 + K; pass B: q heads 2,3 + V
                          m0, m1 = 2 * pas, 2 * pas + 1
                          wk_off = 0 if pas == 0 else 128

                          def mm(acc, k, wsl, st, sp):
                              hsl = hts[k // 4][:, (k % 4) * 512:
                                                (k % 4 + 1) * 512]
                              nc.tensor.matmul(acc[:], wsl(k), hsl,
                                               start=st, stop=sp)

                          wq0 = lambda k, m=m0: weff_sb[
                              :, k * 512 + m * 128: k * 512 + (m + 1) * 128]
                          wq1 = lambda k, m=m1: weff_sb[
                              :, k * 512 + m * 128: k * 512 + (m + 1) * 128]
                          wkv_ = lambda k, o=wk_off: wkv_sb[
                              :, k * 256 + o: k * 256 + o + 128]

                          for step in range(KT + SKEW):
                              if step < KT:
                                  mm(a0, step, wq0, step == 0, step == KT - 1)
                              if step >= SKEW:
                                  k2 = step - SKEW
                                  mm(a1, k2, wq1, k2 == 0, k2 == KT - 1)
                                  mm(a2, k2, wkv_, k2 == 0, k2 == KT - 1)
                              # one interleaved action per step: drain the
                              # previous block's rope pops, and from block 2
                              # on feed qb0's attention between them
                              if step % 2 == 0 and pending:
                                  pending.pop(0)()
                              elif sb_i >= 2:
                                  gen_step()

                          # evacuate this pass; rope/transposes deferred
                          qsl0 = slice(m0 * SB, (m0 + 1) * SB)
                          qsl1 = slice(m1 * SB, (m1 + 1) * SB)
                          nc.scalar.copy(qTs[sb_i][:, qsl0], a0[:])
                          nc.vector.tensor_copy(qTs[sb_i][:, qsl1], a1[:])
                          newpend.append(lambda qsl=qsl0, b=sb_i, fn=fn, sl=sl:
                                         fn(qTs[b], qsl, sl))
                          newpend.append(lambda qsl=qsl1, b=sb_i, fn=fn, sl=sl:
                                         fn(qTs[b], qsl, sl))
                          if pas == 0:
                              nc.vector.tensor_copy(kTs[sb_i][:], a2[:])
                              newpend.append(lambda b=sb_i, fn=fn, sl=sl:
                                             fn(kTs[b], slice(0, SB), sl))
                          else:
                              vtmp = vt.tile([128, SB], BF16, tag="vtmp")
                              nc.scalar.copy(vtmp[:], a2[:])

                              def emit_transposes(sb_i=sb_i, vtmp=vtmp):
                                  # bf16 V transposes via the DMA XBAR: no
                                  # PE/DVE/PSUM involvement at all
                                  for tt in range(4):
                                      nc.sync.dma_start_transpose(
                                          out=vnats[sb_i][:, tt * 128:
                                                          (tt + 1) * 128],
                                          in_=vtmp[:, tt * 128:(tt + 1) * 128],
                                      )
                              newpend.append(emit_transposes)
                      for fnp in pending:   # anything not yet flushed
                          fnp()
                      pending = newpend

                  # qb0 leftovers that didn't fit into blocks 2-3
                  while not genq0_done[0]:
                      gen_step()

              # ============ stage 3+4: qb1..qb3 + output projection ==========
              with (
                  tc.tile_pool(name="psL", bufs=2, space="PSUM") as psL,
                  tc.tile_pool(name="psO", bufs=2, space="PSUM") as psO,
                  tc.tile_pool(name="psD", bufs=2, space="PSUM") as psD,
                  tc.tile_pool(name="psW", bufs=2, space="PSUM") as psW,
              ):
                  def stage4_chunk(qbx, tl, atile, dve_copies=True):
                      # out[q-chunk t, :] partial = attn(:, t-cols) @ Wo
                      t = qbx * 4 + tl
                      obuf = ob.tile([128, D], BF16, tag="obuf")
                      for n4 in range(4):
                          pw = psW.tile([128, SB], F32, tag="pw")
                          for hh in range(HPC):
                              nc.tensor.matmul(
                                  pw[:],
                                  atile[:, hh * SB + tl * 128:
                                        hh * SB + (tl + 1) * 128],
                                  wo_sb[:, hh * D + n4 * SB:
                                        hh * D + (n4 + 1) * SB],
                                  start=(hh == 0), stop=(hh == HPC - 1),
                              )
                          o0 = n4 * SB
                          # split PSUM→SBUF copies between ACT and DVE: ACT
                          # also carries every exp, DVE the normalization
                          if dve_copies and n4 % 2 == 0:
                              nc.vector.tensor_copy(obuf[:, o0:o0 + SB], pw[:])
                          else:
                              nc.scalar.copy(obuf[:, o0:o0 + SB], pw[:])
                          if n4 == 1:
                              nc.sync.dma_start(
                                  out=out[t * 128:(t + 1) * 128, 0:2 * SB],
                                  in_=obuf[:, 0:2 * SB],
                              )
                          elif n4 == 3:
                              nc.sync.dma_start(
                                  out=out[t * 128:(t + 1) * 128, 2 * SB:D],
                                  in_=obuf[:, 2 * SB:D],
                              )

                  prev_attnT = attnT0
                  prev_qb = 0
                  for qi, qb in enumerate([1, 2, 3]):
                      attnT = at.tile([128, HPC * SB], BF16, tag="attnT",
                                      name=f"attnT_{qb}")
                      for ev, h in make_attn_gen(qb, psL, psO, psD, attnT):
                          if ev != "head":
                              continue
                          stage4_chunk(prev_qb, h, prev_attnT)
                          # drain block3's deferred ropes/transposes across
                          # qb1+qb2's heads (qb3 reads them)
                          if qi <= 1 and pending:
                              pending.pop(0)()
                      prev_attnT = attnT
                      prev_qb = qb
                  for tl in range(4):
                      stage4_chunk(prev_qb, tl, prev_attnT, dve_copies=False)
    _split_multiwaits(nc)
    return nc


_NC = None


def _get_nc():
    global _NC
    if _NC is None:
        _NC = build_nc()
    return _NC


def _make_in_maps(hidden, position_ids, Wqa, Wqb, Wk, Wv, Wo):
    hidden = np.asarray(hidden, dtype=np.float32)
    position_ids = np.asarray(position_ids)
    Wqa = np.asarray(Wqa, dtype=np.float32)
    Wqb = np.asarray(Wqb, dtype=np.float32)
    Wk = np.asarray(Wk, dtype=np.float32)
    Wv = np.asarray(Wv, dtype=np.float32)
    Wo = np.asarray(Wo, dtype=np.float32)
    weff_full = Wqa @ Wqb  # [D, H*HD]; exact assoc. fold of the LoRA Q proj

    inv_freq = 1.0 / (ROPE_BASE ** (np.arange(0, ROT, 2, dtype=np.float32) / ROT))
    in_maps = []
    for c in range(N_CORES):
        b, g = c // KVH, c % KVH
        pos = position_ids[b].astype(np.float32)
        freqs = pos[:, None] * inv_freq[None, :]        # [S, 32]
        cosT = np.cos(freqs).T.astype(np.float32)       # [32, S]
        sinT = np.sin(freqs).T.astype(np.float32)
        rcs = np.concatenate([cosT, cosT, sinT, sinT], axis=0).astype(
            ml_dtypes.bfloat16)  # [128, S]
        hsb = (hidden[b].T.reshape(KT, 128, NSB, SB)
               .transpose(1, 2, 0, 3).reshape(128, NSB * KT * SB))
        weff = (weff_full[:, g * HPC * HD:(g + 1) * HPC * HD]
                .reshape(KT, 128, 512).transpose(1, 0, 2).reshape(128, KT * 512))
        wkv = np.concatenate(
            [Wk[:, g * HD:(g + 1) * HD], Wv[:, g * HD:(g + 1) * HD]], axis=1
        ).reshape(KT, 128, 256).transpose(1, 0, 2).reshape(128, KT * 256)
        wog = (Wo[g * HPC * HD:(g + 1) * HPC * HD, :]
               .reshape(HPC, 128, D).transpose(1, 0, 2).reshape(128, HPC * D))
        in_maps.append({
            "hid": np.ascontiguousarray(hsb.astype(ml_dtypes.bfloat16)),
            "weff": np.ascontiguousarray(weff.astype(ml_dtypes.bfloat16)),
            "wkv": np.ascontiguousarray(wkv.astype(ml_dtypes.bfloat16)),
            "wo": np.ascontiguousarray(wog.astype(ml_dtypes.bfloat16)),
            "rcs": np.ascontiguousarray(rcs),
        })
    return in_maps


def _run(inputs, trace=False):
    nc = _get_nc()
    in_maps = _make_in_maps(**inputs)
    res = run_bass_kernel_spmd(nc, in_maps, list(range(N_CORES)), trace=trace)
    out = np.zeros((B, S, D), dtype=np.float32)
    for c in range(N_CORES):
        out[c // KVH] += res.results[c]["out"].astype(np.float32)
    return out, res


def kernel(**inputs) -> np.ndarray:
    return _run(inputs, trace=False)[0]


# revision 49
# speedup vs baseline: 1.0622x; 1.0001x over previous
"""DeepseekV4-style attention (partial-RoPE LoRA-Q GQA sliding-window) on 8
Trainium2 NeuronCores.

Sharding: core c = 4*b + g handles batch b (of 2) and GQA group g (of 4):
q heads 4g..4g+3, kv head g, the matching column slices of Wq_eff/Wk/Wv and
row slice of Wo.  Each core computes a partial output; the host sums the four
partials per batch (in f32; the device ships bf16 partials).

Design notes:
- LoRA Q projection folded on the host (W_eff = Wqa @ Wqb slice).
- Everything downstream of the PSUM accumulators is bf16: qT/kT/vnat/e/attnT.
  bf16 matmuls run 1 cycle/row at any moving-dim size (no fp32r >=256 rule),
  so attention tiles are trimmed exactly; rope runs in the DVE 2x 16-bit mode.
- Stage 1 runs each seq block in TWO passes of 3 PSUM accumulators rotating
  over 4 banks, which leaves 3 PSUM banks free during stage 1.  Those banks
  host a bufs=1 psL/psO/psD set used to compute qb0's attention interleaved
  into blocks 2-3's k-loops: the first attention block rides under stage-1
  matmul backpressure instead of idling the PE on its exp/recip latencies.
- Masking: causal/window staircases span at most 128 cols (one per key
  partition), so affine_select only touches that band.
- V chunks transpose through the DMA XBAR (bf16), not the PE.
- The per-head softmax normalization evacuates po to SBUF immediately (PSUM
  bank frees without waiting on the slow DVE reciprocal).
- Main attention (qb1..qb3) interleaves stage-4 output projection of the
  previous qb after each head, so the PE never drains on DVE tails; the
  final flush projects qb3.
"""

import numpy as np
import ml_dtypes
import concourse.bass as bass
import concourse.mybir as mybir
import concourse.tile as tile
from concourse.bass_utils import run_bass_kernel_spmd

F32 = mybir.dt.float32
F32R = mybir.dt.float32r
BF16 = mybir.dt.bfloat16
ACTF = mybir.ActivationFunctionType
ALU = mybir.AluOpType

B, S, D = 2, 2048, 2048
H, KVH, HD = 16, 4, 128
ROT, LORA, WINDOW = 64, 512, 1024
ROPE_BASE = 10000.0
SCALE = HD ** -0.5

HPC = H // KVH          # 4 q heads per core
SB = 512                # free-dim block for matmuls
NSB = S // SB           # 4 seq blocks
KT = D // 128           # 16 contraction tiles over D
ST = S // 128           # 16 seq 128-chunks
N_CORES = 8
PIPE = 1                # attention chunk software-pipeline depth
SKEW = 2                # stage-1 pass k-skew (late accumulators)


def _split_multiwaits(nc):
    """This image's walrus accepts only one embedded SyncWait per instruction;
    split Tile's multi-wait sync_infos into standalone event-semaphore waits."""
    n = 0
    for func in nc.m.functions:
        for bb in func.blocks:
            insts = list(bb.instructions)
            out = []
            changed = False
            for inst in insts:
                si = inst.sync_info
                if si is not None and si.on_wait and len(si.on_wait) > 1:
                    waits = list(si.on_wait)
                    for w in waits[:-1]:
                        ev = mybir.InstEventSemaphore(
                            name=f"{inst.name}_wsplit_{n}", ins=[], outs=[]
                        )
                        ev.engine = inst.engine
                        ev.sync_info = mybir.SyncInfo(on_wait=[w], on_update=[])
                        out.append(ev)
                        n += 1
                    inst.sync_info = mybir.SyncInfo(
                        on_wait=[waits[-1]], on_update=list(si.on_update or [])
                    )
                    changed = True
                out.append(inst)
            if changed:
                bb.instructions = out
    return n


def build_nc():
    nc = bass.Bass()
    # host-packed layouts: hid col = blk*8192 + k*512 + c; weff col = k*512+c;
    # wkv col = k*256+c; wo col = h*2048+c
    hid = nc.dram_tensor("hid", [128, NSB * KT * SB], BF16, kind="ExternalInput")
    weff = nc.dram_tensor("weff", [128, KT * 512], BF16, kind="ExternalInput")
    wkv = nc.dram_tensor("wkv", [128, KT * 256], BF16, kind="ExternalInput")
    wo = nc.dram_tensor("wo", [128, HPC * D], BF16, kind="ExternalInput")
    rcs = nc.dram_tensor("rcs", [128, S], BF16, kind="ExternalInput")
    out = nc.dram_tensor("out", [S, D], BF16, kind="ExternalOutput")

    with tile.TileContext(nc) as tc:
        with (
            tc.tile_pool(name="cst", bufs=1) as cst,
            tc.tile_pool(name="big", bufs=1) as big,
        ):
            # ---- small constants (engine-built, no DMA) ----
            onesf = cst.tile([128, 128], F32, tag="onesf")
            nc.vector.memset(onesf[:], 1.0)
            ones = cst.tile([128, 128], BF16, tag="ones")
            nc.vector.tensor_copy(ones[:], onesf[:])
            # signed rope permutation P: P[32+i, i] = -1, P[j, 32+j] = +1
            negf = cst.tile([64, 32], F32, tag="negf")
            nc.vector.memset(negf[:], -1.0)
            posf = cst.tile([64, 32], F32, tag="posf")
            nc.vector.memset(posf[:], 1.0)
            permf = cst.tile([64, 64], F32, tag="permf")
            nc.gpsimd.affine_select(
                out=permf[:, 0:32], in_=negf[:], pattern=[[-1, 32]],
                compare_op=ALU.is_equal, fill=0.0, base=-32, channel_multiplier=1,
            )
            nc.gpsimd.affine_select(
                out=permf[:, 32:64], in_=posf[:], pattern=[[-1, 32]],
                compare_op=ALU.is_equal, fill=0.0, base=0, channel_multiplier=1,
            )
            perm = cst.tile([64, 64], BF16, tag="perm")
            nc.vector.tensor_copy(perm[:], permf[:])
            warm = cst.tile([128, 512], BF16, tag="warm")
            nc.vector.memset(warm[:], 0.0)

            # ---- persistent activations, split per seq-block so a late
            # rope/transpose on block b never false-blocks attention reads of
            # earlier blocks (dep tracking is tile-granular) ----
            qTs = [big.tile([128, HPC * SB], BF16, tag=f"qT{b}", name=f"qT{b}")
                   for b in range(NSB)]          # col = h*SB + c
            kTs = [big.tile([128, SB], BF16, tag=f"kT{b}", name=f"kT{b}")
                   for b in range(NSB)]
            vnats = [big.tile([128, SB], BF16, tag=f"vn{b}", name=f"vn{b}")
                     for b in range(NSB)]        # V rows, chunk t at cols (t%4)*128
            wo_sb = big.tile([128, HPC * D], BF16, tag="wo_sb")

            with (
                tc.tile_pool(name="wp", bufs=1) as wp,
                tc.tile_pool(name="hp", bufs=5) as hp,
                tc.tile_pool(name="vt", bufs=2) as vt,
                tc.tile_pool(name="rp", bufs=2) as rp,
                tc.tile_pool(name="at", bufs=2) as at,
                tc.tile_pool(name="ex", bufs=5) as ex,
                tc.tile_pool(name="rc", bufs=2) as rc,
                tc.tile_pool(name="ob", bufs=2) as ob,
            ):
              weff_sb = wp.tile([128, KT * 512], BF16, tag="weff_sb")
              wkv_sb = wp.tile([128, KT * 256], BF16, tag="wkv_sb")
              # bf16 cos/sin: all-bf16 rope operands put the DVE in its
              # 2x 16-bit mode, halving rope cost
              ropeCC = wp.tile([64, S], BF16, tag="ropeCC")
              ropeSS = wp.tile([64, S], BF16, tag="ropeSS")

              def rope_emit_dma(dst, csl, rsl):
                  # PE-free variant for the last block's flush: the half-swap
                  # goes through two small SBUF->SBUF DMAs instead of the
                  # permutation matmul, keeping the PE FIFO clear inside the
                  # main attention loop.
                  swp = rp.tile([64, SB], BF16, tag="swp")
                  nc.sync.dma_start(out=swp[0:32, :], in_=dst[32:64, csl])
                  nc.sync.dma_start(out=swp[32:64, :], in_=dst[0:32, csl])
                  csb = rp.tile([64, SB], BF16, tag="csb2")
                  nc.vector.tensor_mul(csb[:], dst[0:64, csl], ropeCC[:, rsl])
                  tsin = rp.tile([64, SB], BF16, tag="tsin2")
                  nc.vector.tensor_mul(tsin[:], swp[:], ropeSS[:, rsl])
                  nc.vector.tensor_sub(dst[0:32, csl], csb[0:32, :], tsin[0:32, :])
                  nc.vector.tensor_add(dst[32:64, csl], csb[32:64, :], tsin[32:64, :])

              def make_attn_gen(qb, psLp, psOp, psDp, atiles):
                  # generator emitting one qb's attention; yields
                  # ("chunk", h) after each chunk and ("head", h) after each
                  # head's normalization tail, so the caller paces emission.
                  # atiles: per-head attnT tiles — per-head granularity lets
                  # stage-4 accumulate heads 0..2 before head 3 normalizes.
                  def gen():
                      q0 = qb * SB
                      kt_lo = max(0, q0 - WINDOW + 1) // 128
                      kt_hi = q0 // 128 + 3
                      for h in range(HPC):
                          po = psOp.tile([128, SB], F32, tag="po")
                          pd = psDp.tile([128, SB], F32, tag="pd")
                          inflight = []

                          def flush_one(po=po, pd=pd, inflight=inflight,
                                        kt_lo=kt_lo, kt_hi=kt_hi):
                              kt, lo, hi, e = inflight.pop(0)
                              kb, kc = kt // 4, (kt % 4) * 128
                              st, sp = (kt == kt_lo), (kt == kt_hi)
                              nc.tensor.matmul(
                                  po[:, lo:hi], vnats[kb][:, kc:kc + 128],
                                  e[:, lo:hi], start=st, stop=sp,
                              )
                              nc.tensor.matmul(
                                  pd[:, lo:hi], ones[:], e[:, lo:hi],
                                  start=st, stop=sp,
                              )

                          for kt in range(kt_lo, kt_hi + 1):
                              dp = kt * 128 - q0
                              # exact valid cols [lo, hi): bf16 matmuls have no
                              # fp32r moving-dim constraint, so no padding
                              lo = max(0, dp)
                              hi = min(SB, dp + WINDOW + 128)
                              kb, kc = kt // 4, (kt % 4) * 128
                              pl = psLp.tile([128, SB], F32, tag="pl")
                              nc.tensor.matmul(
                                  pl[:, lo:hi], kTs[kb][:, kc:kc + 128],
                                  qTs[qb][:, h * SB + lo: h * SB + hi],
                                  start=True, stop=True,
                              )
                              e = ex.tile([128, SB], BF16, tag="e")
                              nc.scalar.activation(
                                  e[:, lo:hi], pl[:, lo:hi], ACTF.Exp, scale=SCALE
                              )
                              # the mask staircase spans at most 128 cols (one
                              # per key partition) — select only on that band
                              if dp >= 0:
                                  n2 = min(128, hi - lo)
                                  nc.gpsimd.affine_select(
                                      out=e[:, lo:lo + n2], in_=e[:, lo:lo + n2],
                                      pattern=[[1, n2]], compare_op=ALU.is_ge,
                                      fill=0.0, base=lo - dp,
                                      channel_multiplier=-1,
                                  )
                              elif dp <= -(WINDOW - SB + 1):
                                  l2 = max(lo, hi - 128)
                                  nc.gpsimd.affine_select(
                                      out=e[:, l2:hi], in_=e[:, l2:hi],
                                      pattern=[[-1, hi - l2]],
                                      compare_op=ALU.is_ge,
                                      fill=0.0, base=WINDOW - 1 + dp - l2,
                                      channel_multiplier=1,
                                  )
                              inflight.append((kt, lo, hi, e))
                              if len(inflight) > PIPE:
                                  flush_one()
                              yield ("chunk", h)
                          while inflight:
                              flush_one()
                          # evacuate po to SBUF right away so its PSUM bank
                          # frees without waiting for the slow reciprocal
                          poS = rc.tile([128, SB], BF16, tag="poS")
                          nc.vector.tensor_copy(poS[:], po[:])
                          rec = rc.tile([128, SB], F32, tag="rec")
                          nc.vector.reciprocal(rec[:], pd[:])
                          nc.vector.tensor_mul(atiles[h][:], poS[:], rec[:])
                          yield ("head", h)
                  return gen()

              # ================= stage 1 (+ overlapped qb0) =================
              with (
                  tc.tile_pool(name="psA", bufs=1, space="PSUM") as psA,
                  tc.tile_pool(name="psP", bufs=1, space="PSUM") as psP,
                  tc.tile_pool(name="psL0", bufs=1, space="PSUM") as psL0,
                  tc.tile_pool(name="psO0", bufs=1, space="PSUM") as psO0,
                  tc.tile_pool(name="psD0", bufs=1, space="PSUM") as psD0,
              ):
                  # PE warmup: dependency-free matmuls ramp the PE p-state and
                  # keep it busy while the first weight/hidden DMAs land.
                  for _ in range(18):
                      pPw = psP.tile([64, SB], F32, tag="pP")
                      nc.tensor.matmul(pPw[:], perm[:], warm[0:64, :],
                                       start=True, stop=True)

                  def dma_w(g0, g1):
                      # SP queue: DMA issue on the ACT sequencer (667ns each)
                      # delays the pass-evacuation copies at startup
                      nc.sync.dma_start(
                          out=weff_sb[:, g0 * 512:g1 * 512],
                          in_=weff[:, g0 * 512:g1 * 512],
                      )
                      nc.sync.dma_start(
                          out=wkv_sb[:, g0 * 256:g1 * 256],
                          in_=wkv[:, g0 * 256:g1 * 256],
                      )

                  def dma_ht(sb_i, qd):
                      t = hp.tile([128, 2048], BF16, tag="ht",
                                  name=f"ht_{sb_i}_{qd}")
                      nc.sync.dma_start(
                          out=t[:],
                          in_=hid[:, sb_i * 8192 + qd * 2048:
                                  sb_i * 8192 + (qd + 1) * 2048],
                      )
                      return t

                  # startup interleave: weight k-groups racing block-0 hidden
                  h0t = []
                  dma_w(0, 1)
                  t00 = hp.tile([128, 2048], BF16, tag="ht", name="ht_0_0")
                  nc.sync.dma_start(out=t00[:, 0:512], in_=hid[:, 0:512])
                  nc.sync.dma_start(out=t00[:, 512:1024], in_=hid[:, 512:1024])
                  h0t.append(t00)
                  dma_w(1, 4)
                  nc.sync.dma_start(out=t00[:, 1024:2048], in_=hid[:, 1024:2048])
                  h0t.append(dma_ht(0, 1))
                  dma_w(4, 8)
                  h0t.append(dma_ht(0, 2))
                  dma_w(8, 12)
                  h0t.append(dma_ht(0, 3))
                  dma_w(12, 16)
                  nc.sync.dma_start(out=ropeCC[:], in_=rcs[0:64, :])
                  nc.sync.dma_start(out=ropeSS[:], in_=rcs[64:128, :])

                  def rope_emit(dst, csl, rsl):
                      # dst rows 0:64 hold [x1; x2]; out = x*cos + P^T(x*sin)
                      tsin = rp.tile([64, SB], BF16, tag="tsin")
                      nc.vector.tensor_mul(tsin[:], dst[0:64, csl], ropeSS[:, rsl])
                      csb = rp.tile([64, SB], BF16, tag="csb")
                      nc.vector.tensor_mul(csb[:], dst[0:64, csl], ropeCC[:, rsl])
                      pP = psP.tile([64, SB], F32, tag="pP")
                      nc.tensor.matmul(pP[:], perm[:], tsin[:], start=True,
                                       stop=True)
                      nc.vector.tensor_add(dst[0:64, csl], csb[:], pP[:])

                  # qb0's attention, interleaved into blocks 2-3 below
                  atiles0 = [at.tile([128, SB], BF16, tag=f"aT{h}",
                                     name=f"attnT_0_h{h}") for h in range(HPC)]
                  genq0 = make_attn_gen(0, psL0, psO0, psD0, atiles0)
                  genq0_done = [False]

                  def gen_step():
                      if not genq0_done[0] and next(genq0, None) is None:
                          genq0_done[0] = True

                  # 4 rotating PSUM accumulators; each pass takes 3, frees 3.
                  # Taking from the left reuses the longest-evacuated banks.
                  ring = [psA.tile([128, SB], F32, tag=f"acc{i}",
                                   name=f"acc{i}") for i in range(4)]

                  pending = []   # prev block's deferred rope/transpose pops
                  for sb_i in range(NSB):
                      sl = slice(sb_i * SB, (sb_i + 1) * SB)
                      if sb_i > 0:
                          hts = [dma_ht(sb_i, qd) for qd in range(4)]
                      else:
                          hts = h0t
                      if sb_i == 2:
                          # Wo prefetch off the critical startup queue: lone
                          # DMA on the otherwise-idle ACT queue, ~60us before
                          # the first stage-4 needs it
                          nc.scalar.dma_start(out=wo_sb[:], in_=wo[:, :])
                      newpend = []
                      last = sb_i == NSB - 1
                      fn = rope_emit_dma if last else rope_emit
                      for pas in range(2):
                          a0, a1, a2 = ring[0], ring[1], ring[2]
                          ring = ring[3:] + [a0, a1, a2]
                          # pass A: q heads 0,1 + K; pass B: q heads 2,3 + V
                          m0, m1 = 2 * pas, 2 * pas + 1
                          wk_off = 0 if pas == 0 else 128

                          def mm(acc, k, wsl, st, sp):
                              hsl = hts[k // 4][:, (k % 4) * 512:
                                                (k % 4 + 1) * 512]
                              nc.tensor.matmul(acc[:], wsl(k), hsl,
                                               start=st, stop=sp)

                          wq0 = lambda k, m=m0: weff_sb[
                              :, k * 512 + m * 128: k * 512 + (m + 1) * 128]
                          wq1 = lambda k, m=m1: weff_sb[
                              :, k * 512 + m * 128: k * 512 + (m + 1) * 128]
                          wkv_ = lambda k, o=wk_off: wkv_sb[
                              :, k * 256 + o: k * 256 + o + 128]

                          for step in range(KT + SKEW):
                              if step < KT:
                                  mm(a0, step, wq0, step == 0, step == KT - 1)
                              if step >= SKEW:
                                  k2 = step - SKEW
                                  mm(a1, k2, wq1, k2 == 0, k2 == KT - 1)
                                  mm(a2, k2, wkv_, k2 == 0, k2 == KT - 1)
                              # one interleaved action per step: drain the
                              # previous block's rope pops, and from block 2
                              # on feed qb0's attention between them
                              if step % 2 == 0 and pending:
                                  pending.pop(0)()
                              elif sb_i >= 2:
                                  gen_step()

                          # evacuate this pass; rope/transposes deferred
                          qsl0 = slice(m0 * SB, (m0 + 1) * SB)
                          qsl1 = slice(m1 * SB, (m1 + 1) * SB)
                          nc.scalar.copy(qTs[sb_i][:, qsl0], a0[:])
                          nc.vector.tensor_copy(qTs[sb_i][:, qsl1], a1[:])
                          newpend.append(lambda qsl=qsl0, b=sb_i, fn=fn, sl=sl:
                                         fn(qTs[b], qsl, sl))
                          newpend.append(lambda qsl=qsl1, b=sb_i, fn=fn, sl=sl:
                                         fn(qTs[b], qsl, sl))
                          if pas == 0:
                              nc.vector.tensor_copy(kTs[sb_i][:], a2[:])
                              newpend.append(lambda b=sb_i, fn=fn, sl=sl:
                                             fn(kTs[b], slice(0, SB), sl))
                          else:
                              vtmp = vt.tile([128, SB], BF16, tag="vtmp")
                              nc.scalar.copy(vtmp[:], a2[:])

                              def emit_transposes(sb_i=sb_i, vtmp=vtmp):
                                  # bf16 V transposes via the DMA XBAR: no
                                  # PE/DVE/PSUM involvement at all
                                  for tt in range(4):
                                      nc.sync.dma_start_transpose(
                                          out=vnats[sb_i][:, tt * 128:
                                                          (tt + 1) * 128],
                                          in_=vtmp[:, tt * 128:(tt + 1) * 128],
                                      )
                              newpend.append(emit_transposes)
                      for fnp in pending:   # anything not yet flushed
                          fnp()
                      pending = newpend

                  # qb0 leftovers that didn't fit into blocks 2-3
                  while not genq0_done[0]:
                      gen_step()

              # ============ stage 3+4: qb1..qb3 + output projection ==========
              with (
                  tc.tile_pool(name="psL", bufs=2, space="PSUM") as psL,
                  tc.tile_pool(name="psO", bufs=2, space="PSUM") as psO,
                  tc.tile_pool(name="psD", bufs=2, space="PSUM") as psD,
                  tc.tile_pool(name="psW", bufs=2, space="PSUM") as psW,
              ):
                  def stage4_chunk(qbx, tl, atiles, dve_copies=True):
                      # out[q-chunk t, :] partial = attn(:, t-cols) @ Wo
                      t = qbx * 4 + tl
                      obuf = ob.tile([128, D], BF16, tag="obuf")
                      for n4 in range(4):
                          pw = psW.tile([128, SB], F32, tag="pw")
                          for hh in range(HPC):
                              nc.tensor.matmul(
                                  pw[:],
                                  atiles[hh][:, tl * 128:(tl + 1) * 128],
                                  wo_sb[:, hh * D + n4 * SB:
                                        hh * D + (n4 + 1) * SB],
                                  start=(hh == 0), stop=(hh == HPC - 1),
                              )
                          o0 = n4 * SB
                          # split PSUM→SBUF copies between ACT and DVE: ACT
                          # also carries every exp, DVE the normalization
                          if dve_copies and n4 % 2 == 0:
                              nc.vector.tensor_copy(obuf[:, o0:o0 + SB], pw[:])
                          else:
                              nc.scalar.copy(obuf[:, o0:o0 + SB], pw[:])
                          if n4 == 1:
                              nc.sync.dma_start(
                                  out=out[t * 128:(t + 1) * 128, 0:2 * SB],
                                  in_=obuf[:, 0:2 * SB],
                              )
                          elif n4 == 3:
                              nc.sync.dma_start(
                                  out=out[t * 128:(t + 1) * 128, 2 * SB:D],
                                  in_=obuf[:, 2 * SB:D],
                              )

                  prev_at = atiles0
                  prev_qb = 0
                  for qi, qb in enumerate([1, 2, 3]):
                      atiles = [at.tile([128, SB], BF16, tag=f"aT{h}",
                                        name=f"attnT_{qb}_h{h}")
                                for h in range(HPC)]
                      for ev, h in make_attn_gen(qb, psL, psO, psD, atiles):
                          if ev != "head":
                              continue
                          stage4_chunk(prev_qb, h, prev_at)
                          # drain block3's deferred ropes/transposes across
                          # qb1+qb2's heads (qb3 reads them)
                          if qi <= 1 and pending:
                              pending.pop(0)()
                      prev_at = atiles
                      prev_qb = qb
                  for tl in range(4):
                      stage4_chunk(prev_qb, tl, prev_at, dve_copies=False)
    _split_multiwaits(nc)
    return nc


_NC = None


def _get_nc():
    global _NC
    if _NC is None:
        _NC = build_nc()
    return _NC


def _make_in_maps(hidden, position_ids, Wqa, Wqb, Wk, Wv, Wo):
    hidden = np.asarray(hidden, dtype=np.float32)
    position_ids = np.asarray(position_ids)
    Wqa = np.asarray(Wqa, dtype=np.float32)
    Wqb = np.asarray(Wqb, dtype=np.float32)
    Wk = np.asarray(Wk, dtype=np.float32)
    Wv = np.asarray(Wv, dtype=np.float32)
    Wo = np.asarray(Wo, dtype=np.float32)
    weff_full = Wqa @ Wqb  # [D, H*HD]; exact assoc. fold of the LoRA Q proj

    inv_freq = 1.0 / (ROPE_BASE ** (np.arange(0, ROT, 2, dtype=np.float32) / ROT))
    in_maps = []
    for c in range(N_CORES):
        b, g = c // KVH, c % KVH
        pos = position_ids[b].astype(np.float32)
        freqs = pos[:, None] * inv_freq[None, :]        # [S, 32]
        cosT = np.cos(freqs).T.astype(np.float32)       # [32, S]
        sinT = np.sin(freqs).T.astype(np.float32)
        rcs = np.concatenate([cosT, cosT, sinT, sinT], axis=0).astype(
            ml_dtypes.bfloat16)  # [128, S]
        hsb = (hidden[b].T.reshape(KT, 128, NSB, SB)
               .transpose(1, 2, 0, 3).reshape(128, NSB * KT * SB))
        weff = (weff_full[:, g * HPC * HD:(g + 1) * HPC * HD]
                .reshape(KT, 128, 512).transpose(1, 0, 2).reshape(128, KT * 512))
        wkv = np.concatenate(
            [Wk[:, g * HD:(g + 1) * HD], Wv[:, g * HD:(g + 1) * HD]], axis=1
        ).reshape(KT, 128, 256).transpose(1, 0, 2).reshape(128, KT * 256)
        wog = (Wo[g * HPC * HD:(g + 1) * HPC * HD, :]
               .reshape(HPC, 128, D).transpose(1, 0, 2).reshape(128, HPC * D))
        in_maps.append({
            "hid": np.ascontiguousarray(hsb.astype(ml_dtypes.bfloat16)),
            "weff": np.ascontiguousarray(weff.astype(ml_dtypes.bfloat16)),
            "wkv": np.ascontiguousarray(wkv.astype(ml_dtypes.bfloat16)),
            "wo": np.ascontiguousarray(wog.astype(ml_dtypes.bfloat16)),
            "rcs": np.ascontiguousarray(rcs),
        })
    return in_maps


def _run(inputs, trace=False):
    nc = _get_nc()
    in_maps = _make_in_maps(**inputs)
    res = run_bass_kernel_spmd(nc, in_maps, list(range(N_CORES)), trace=trace)
    out = np.zeros((B, S, D), dtype=np.float32)
    for c in range(N_CORES):
        out[c // KVH] += res.results[c]["out"].astype(np.float32)
    return out, res


def kernel(**inputs) -> np.ndarray:
    return _run(inputs, trace=False)[0]


# revision 57
# speedup vs baseline: 1.1059x; 1.0411x over previous
"""DeepseekV4-style attention (partial-RoPE LoRA-Q GQA sliding-window) on 8
Trainium2 NeuronCores.

Sharding: core c = 4*b + g handles batch b (of 2) and GQA group g (of 4):
q heads 4g..4g+3, kv head g, the matching column slices of Wq_eff/Wk/Wv and
row slice of Wo.  Each core computes a partial output; the host sums the four
partials per batch (in f32; the device ships bf16 partials).

Design notes:
- LoRA Q projection folded on the host (W_eff = Wqa @ Wqb slice).
- Everything downstream of the PSUM accumulators is bf16: qT/kT/vnat/e/attnT.
  bf16 matmuls run 1 cycle/row at any moving-dim size (no fp32r >=256 rule),
  so attention tiles are trimmed exactly; rope runs in the DVE 2x 16-bit mode.
- Stage 1 runs each seq block in TWO passes of 3 PSUM accumulators rotating
  over 4 banks, which leaves 3 PSUM banks free during stage 1.  Those banks
  host a bufs=1 psL/psO/psD set used to compute qb0's attention interleaved
  into blocks 2-3's k-loops: the first attention block rides under stage-1
  matmul backpressure instead of idling the PE on its exp/recip latencies.
- Masking: causal/window staircases span at most 128 cols (one per key
  partition), so affine_select only touches that band.
- V chunks transpose through the DMA XBAR (bf16), not the PE.
- The per-head softmax normalization evacuates po to SBUF immediately (PSUM
  bank frees without waiting on the slow DVE reciprocal).
- Main attention (qb1..qb3) interleaves stage-4 output projection of the
  previous qb after each head, so the PE never drains on DVE tails; the
  final flush projects qb3.
"""

import numpy as np
import ml_dtypes
import concourse.bass as bass
import concourse.mybir as mybir
import concourse.tile as tile
from concourse.bass_utils import run_bass_kernel_spmd

F32 = mybir.dt.float32
F32R = mybir.dt.float32r
BF16 = mybir.dt.bfloat16
ACTF = mybir.ActivationFunctionType
ALU = mybir.AluOpType

B, S, D = 2, 2048, 2048
H, KVH, HD = 16, 4, 128
ROT, LORA, WINDOW = 64, 512, 1024
ROPE_BASE = 10000.0
SCALE = HD ** -0.5

HPC = H // KVH          # 4 q heads per core
SB = 512                # free-dim block for matmuls
NSB = S // SB           # 4 seq blocks
KT = D // 128           # 16 contraction tiles over D
ST = S // 128           # 16 seq 128-chunks
N_CORES = 8
PIPE = 1                # attention chunk software-pipeline depth
SKEW = 2                # stage-1 pass k-skew (late accumulators)


def _split_multiwaits(nc):
    """This image's walrus accepts only one embedded SyncWait per instruction;
    split Tile's multi-wait sync_infos into standalone event-semaphore waits."""
    n = 0
    for func in nc.m.functions:
        for bb in func.blocks:
            insts = list(bb.instructions)
            out = []
            changed = False
            for inst in insts:
                si = inst.sync_info
                if si is not None and si.on_wait and len(si.on_wait) > 1:
                    waits = list(si.on_wait)
                    for w in waits[:-1]:
                        ev = mybir.InstEventSemaphore(
                            name=f"{inst.name}_wsplit_{n}", ins=[], outs=[]
                        )
                        ev.engine = inst.engine
                        ev.sync_info = mybir.SyncInfo(on_wait=[w], on_update=[])
                        out.append(ev)
                        n += 1
                    inst.sync_info = mybir.SyncInfo(
                        on_wait=[waits[-1]], on_update=list(si.on_update or [])
                    )
                    changed = True
                out.append(inst)
            if changed:
                bb.instructions = out
    return n


def build_nc():
    nc = bass.Bass()
    # host-packed layouts: hid col = blk*8192 + k*512 + c; weff col = k*512+c;
    # wkv col = k*256+c; wo col = h*2048+c
    hid = nc.dram_tensor("hid", [128, NSB * KT * SB], BF16, kind="ExternalInput")
    weff = nc.dram_tensor("weff", [128, KT * 512], BF16, kind="ExternalInput")
    wkv = nc.dram_tensor("wkv", [128, KT * 256], BF16, kind="ExternalInput")
    wo = nc.dram_tensor("wo", [128, HPC * D], BF16, kind="ExternalInput")
    rcs = nc.dram_tensor("rcs", [128, S], BF16, kind="ExternalInput")
    out = nc.dram_tensor("out", [S, D], BF16, kind="ExternalOutput")

    with tile.TileContext(nc) as tc:
        with (
            tc.tile_pool(name="cst", bufs=1) as cst,
            tc.tile_pool(name="big", bufs=1) as big,
        ):
            # ---- small constants (engine-built, no DMA) ----
            onesf = cst.tile([128, 128], F32, tag="onesf")
            nc.vector.memset(onesf[:], 1.0)
            ones = cst.tile([128, 128], BF16, tag="ones")
            nc.vector.tensor_copy(ones[:], onesf[:])
            # signed rope permutation P: P[32+i, i] = -1, P[j, 32+j] = +1
            negf = cst.tile([64, 32], F32, tag="negf")
            nc.vector.memset(negf[:], -1.0)
            posf = cst.tile([64, 32], F32, tag="posf")
            nc.vector.memset(posf[:], 1.0)
            permf = cst.tile([64, 64], F32, tag="permf")
            nc.gpsimd.affine_select(
                out=permf[:, 0:32], in_=negf[:], pattern=[[-1, 32]],
                compare_op=ALU.is_equal, fill=0.0, base=-32, channel_multiplier=1,
            )
            nc.gpsimd.affine_select(
                out=permf[:, 32:64], in_=posf[:], pattern=[[-1, 32]],
                compare_op=ALU.is_equal, fill=0.0, base=0, channel_multiplier=1,
            )
            perm = cst.tile([64, 64], BF16, tag="perm")
            nc.vector.tensor_copy(perm[:], permf[:])
            warm = cst.tile([128, 512], BF16, tag="warm")
            nc.vector.memset(warm[:], 0.0)

            # ---- persistent activations, split per seq-block so a late
            # rope/transpose on block b never false-blocks attention reads of
            # earlier blocks (dep tracking is tile-granular) ----
            qTs = [big.tile([128, HPC * SB], BF16, tag=f"qT{b}", name=f"qT{b}")
                   for b in range(NSB)]          # col = h*SB + c
            kTs = [big.tile([128, SB], BF16, tag=f"kT{b}", name=f"kT{b}")
                   for b in range(NSB)]
            vnats = [big.tile([128, SB], BF16, tag=f"vn{b}", name=f"vn{b}")
                     for b in range(NSB)]        # V rows, chunk t at cols (t%4)*128
            wo_sb = big.tile([128, HPC * D], BF16, tag="wo_sb")

            with (
                tc.tile_pool(name="wp", bufs=1) as wp,
                tc.tile_pool(name="hp", bufs=5) as hp,
                tc.tile_pool(name="vt", bufs=2) as vt,
                tc.tile_pool(name="rp", bufs=2) as rp,
                tc.tile_pool(name="at", bufs=2) as at,
                tc.tile_pool(name="ex", bufs=5) as ex,
                tc.tile_pool(name="rc", bufs=2) as rc,
                tc.tile_pool(name="ob", bufs=2) as ob,
            ):
              weff_sb = wp.tile([128, KT * 512], BF16, tag="weff_sb")
              wkv_sb = wp.tile([128, KT * 256], BF16, tag="wkv_sb")
              # bf16 cos/sin: all-bf16 rope operands put the DVE in its
              # 2x 16-bit mode, halving rope cost
              ropeCC = wp.tile([64, S], BF16, tag="ropeCC")
              ropeSS = wp.tile([64, S], BF16, tag="ropeSS")

              def rope_emit_dma(dst, csl, rsl):
                  # PE-free variant for the last block's flush: the half-swap
                  # goes through two small SBUF->SBUF DMAs instead of the
                  # permutation matmul, keeping the PE FIFO clear inside the
                  # main attention loop.
                  swp = rp.tile([64, SB], BF16, tag="swp")
                  nc.sync.dma_start(out=swp[0:32, :], in_=dst[32:64, csl])
                  nc.sync.dma_start(out=swp[32:64, :], in_=dst[0:32, csl])
                  csb = rp.tile([64, SB], BF16, tag="csb2")
                  nc.vector.tensor_mul(csb[:], dst[0:64, csl], ropeCC[:, rsl])
                  tsin = rp.tile([64, SB], BF16, tag="tsin2")
                  nc.vector.tensor_mul(tsin[:], swp[:], ropeSS[:, rsl])
                  nc.vector.tensor_sub(dst[0:32, csl], csb[0:32, :], tsin[0:32, :])
                  nc.vector.tensor_add(dst[32:64, csl], csb[32:64, :], tsin[32:64, :])

              def make_attn_gen(qb, psLp, psOp, psDp, atiles, split3=None):
                  # generator emitting one qb's attention; yields
                  # ("chunk", h) after each chunk and ("head", h) after each
                  # head's normalization tail, so the caller paces emission.
                  # atiles: per-head attnT tiles — per-head granularity lets
                  # stage-4 accumulate heads 0..2 before head 3 normalizes.
                  # split3: optional 4x [128,128] tiles for the last head —
                  # normalization lands in 128-col slices so the final flush
                  # chunk t only waits for its own slice.
                  def gen():
                      q0 = qb * SB
                      kt_lo = max(0, q0 - WINDOW + 1) // 128
                      kt_hi = q0 // 128 + 3
                      for h in range(HPC):
                          po = psOp.tile([128, SB], F32, tag="po")
                          pd = psDp.tile([128, SB], F32, tag="pd")
                          inflight = []

                          def flush_one(po=po, pd=pd, inflight=inflight,
                                        kt_lo=kt_lo, kt_hi=kt_hi):
                              kt, lo, hi, e = inflight.pop(0)
                              kb, kc = kt // 4, (kt % 4) * 128
                              st, sp = (kt == kt_lo), (kt == kt_hi)
                              nc.tensor.matmul(
                                  po[:, lo:hi], vnats[kb][:, kc:kc + 128],
                                  e[:, lo:hi], start=st, stop=sp,
                              )
                              nc.tensor.matmul(
                                  pd[:, lo:hi], ones[:], e[:, lo:hi],
                                  start=st, stop=sp,
                              )

                          for kt in range(kt_lo, kt_hi + 1):
                              dp = kt * 128 - q0
                              # exact valid cols [lo, hi): bf16 matmuls have no
                              # fp32r moving-dim constraint, so no padding
                              lo = max(0, dp)
                              hi = min(SB, dp + WINDOW + 128)
                              kb, kc = kt // 4, (kt % 4) * 128
                              pl = psLp.tile([128, SB], F32, tag="pl")
                              nc.tensor.matmul(
                                  pl[:, lo:hi], kTs[kb][:, kc:kc + 128],
                                  qTs[qb][:, h * SB + lo: h * SB + hi],
                                  start=True, stop=True,
                              )
                              e = ex.tile([128, SB], BF16, tag="e")
                              nc.scalar.activation(
                                  e[:, lo:hi], pl[:, lo:hi], ACTF.Exp, scale=SCALE
                              )
                              # the mask staircase spans at most 128 cols (one
                              # per key partition) — select only on that band
                              if dp >= 0:
                                  n2 = min(128, hi - lo)
                                  nc.gpsimd.affine_select(
                                      out=e[:, lo:lo + n2], in_=e[:, lo:lo + n2],
                                      pattern=[[1, n2]], compare_op=ALU.is_ge,
                                      fill=0.0, base=lo - dp,
                                      channel_multiplier=-1,
                                  )
                              elif dp <= -(WINDOW - SB + 1):
                                  l2 = max(lo, hi - 128)
                                  nc.gpsimd.affine_select(
                                      out=e[:, l2:hi], in_=e[:, l2:hi],
                                      pattern=[[-1, hi - l2]],
                                      compare_op=ALU.is_ge,
                                      fill=0.0, base=WINDOW - 1 + dp - l2,
                                      channel_multiplier=1,
                                  )
                              inflight.append((kt, lo, hi, e))
                              if len(inflight) > PIPE:
                                  flush_one()
                              yield ("chunk", h)
                          while inflight:
                              flush_one()
                          # evacuate po to SBUF right away so its PSUM bank
                          # frees without waiting for the slow reciprocal
                          poS = rc.tile([128, SB], BF16, tag="poS")
                          nc.vector.tensor_copy(poS[:], po[:])
                          if split3 is not None and h == HPC - 1:
                              for tq in range(4):
                                  c2 = slice(tq * 128, (tq + 1) * 128)
                                  recq = rc.tile([128, 128], F32, tag="recq")
                                  nc.vector.reciprocal(recq[:], pd[:, c2])
                                  nc.vector.tensor_mul(
                                      split3[tq][:], poS[:, c2], recq[:]
                                  )
                          else:
                              rec = rc.tile([128, SB], F32, tag="rec")
                              nc.vector.reciprocal(rec[:], pd[:])
                              nc.vector.tensor_mul(atiles[h][:], poS[:], rec[:])
                          yield ("head", h)
                  return gen()

              # ================= stage 1 (+ overlapped qb0) =================
              with (
                  tc.tile_pool(name="psA", bufs=1, space="PSUM") as psA,
                  tc.tile_pool(name="psP", bufs=1, space="PSUM") as psP,
                  tc.tile_pool(name="psL0", bufs=1, space="PSUM") as psL0,
                  tc.tile_pool(name="psO0", bufs=1, space="PSUM") as psO0,
                  tc.tile_pool(name="psD0", bufs=1, space="PSUM") as psD0,
              ):
                  # PE warmup: dependency-free matmuls ramp the PE p-state and
                  # keep it busy while the first weight/hidden DMAs land.
                  for _ in range(18):
                      pPw = psP.tile([64, SB], F32, tag="pP")
                      nc.tensor.matmul(pPw[:], perm[:], warm[0:64, :],
                                       start=True, stop=True)

                  def dma_w(g0, g1):
                      # weights ride the Activation HWDGE queue so the SP
                      # queue can stream block-0 hidden without serialization
                      nc.scalar.dma_start(
                          out=weff_sb[:, g0 * 512:g1 * 512],
                          in_=weff[:, g0 * 512:g1 * 512],
                      )
                      nc.scalar.dma_start(
                          out=wkv_sb[:, g0 * 256:g1 * 256],
                          in_=wkv[:, g0 * 256:g1 * 256],
                      )

                  def dma_ht(sb_i, qd):
                      t = hp.tile([128, 2048], BF16, tag="ht",
                                  name=f"ht_{sb_i}_{qd}")
                      nc.sync.dma_start(
                          out=t[:],
                          in_=hid[:, sb_i * 8192 + qd * 2048:
                                  sb_i * 8192 + (qd + 1) * 2048],
                      )
                      return t

                  # startup interleave: weight k-groups racing block-0 hidden
                  h0t = []
                  dma_w(0, 1)
                  t00 = hp.tile([128, 2048], BF16, tag="ht", name="ht_0_0")
                  nc.sync.dma_start(out=t00[:, 0:512], in_=hid[:, 0:512])
                  nc.sync.dma_start(out=t00[:, 512:1024], in_=hid[:, 512:1024])
                  h0t.append(t00)
                  dma_w(1, 4)
                  nc.sync.dma_start(out=t00[:, 1024:2048], in_=hid[:, 1024:2048])
                  h0t.append(dma_ht(0, 1))
                  dma_w(4, 8)
                  h0t.append(dma_ht(0, 2))
                  dma_w(8, 12)
                  h0t.append(dma_ht(0, 3))
                  dma_w(12, 16)
                  nc.scalar.dma_start(out=ropeCC[:], in_=rcs[0:64, :])
                  nc.scalar.dma_start(out=ropeSS[:], in_=rcs[64:128, :])

                  def rope_emit(dst, csl, rsl):
                      # dst rows 0:64 hold [x1; x2]; out = x*cos + P^T(x*sin)
                      tsin = rp.tile([64, SB], BF16, tag="tsin")
                      nc.vector.tensor_mul(tsin[:], dst[0:64, csl], ropeSS[:, rsl])
                      csb = rp.tile([64, SB], BF16, tag="csb")
                      nc.vector.tensor_mul(csb[:], dst[0:64, csl], ropeCC[:, rsl])
                      pP = psP.tile([64, SB], F32, tag="pP")
                      nc.tensor.matmul(pP[:], perm[:], tsin[:], start=True,
                                       stop=True)
                      nc.vector.tensor_add(dst[0:64, csl], csb[:], pP[:])

                  # qb0's attention, interleaved into blocks 2-3 below
                  atiles0 = [at.tile([128, SB], BF16, tag=f"aT{h}",
                                     name=f"attnT_0_h{h}") for h in range(HPC)]
                  genq0 = make_attn_gen(0, psL0, psO0, psD0, atiles0)
                  genq0_done = [False]
                  cooldown = [0]

                  def gen_step():
                      if genq0_done[0]:
                          return None
                      ev = next(genq0, None)
                      if ev is None:
                          genq0_done[0] = True
                      return ev

                  # 4 rotating PSUM accumulators; each pass takes 3, frees 3.
                  # Taking from the left reuses the longest-evacuated banks.
                  ring = [psA.tile([128, SB], F32, tag=f"acc{i}",
                                   name=f"acc{i}") for i in range(4)]

                  pending = []   # prev block's deferred rope/transpose pops
                  for sb_i in range(NSB):
                      sl = slice(sb_i * SB, (sb_i + 1) * SB)
                      if sb_i > 0:
                          hts = [dma_ht(sb_i, qd) for qd in range(4)]
                      else:
                          hts = h0t
                      if sb_i == 2:
                          # Wo prefetch off the critical startup queue: lone
                          # DMA on the otherwise-idle ACT queue, ~60us before
                          # the first stage-4 needs it
                          nc.scalar.dma_start(out=wo_sb[:], in_=wo[:, :])
                      newpend = []
                      last = sb_i == NSB - 1
                      fn = rope_emit_dma if last else rope_emit
                      for pas in range(2):
                          a0, a1, a2 = ring[0], ring[1], ring[2]
                          ring = ring[3:] + [a0, a1, a2]
                          # pass A: q heads 0,1 + K; pass B: q heads 2,3 + V
                          m0, m1 = 2 * pas, 2 * pas + 1
                          wk_off = 0 if pas == 0 else 128

                          def mm(acc, k, wsl, st, sp):
                              hsl = hts[k // 4][:, (k % 4) * 512:
                                                (k % 4 + 1) * 512]
                              nc.tensor.matmul(acc[:], wsl(k), hsl,
                                               start=st, stop=sp)

                          wq0 = lambda k, m=m0: weff_sb[
                              :, k * 512 + m * 128: k * 512 + (m + 1) * 128]
                          wq1 = lambda k, m=m1: weff_sb[
                              :, k * 512 + m * 128: k * 512 + (m + 1) * 128]
                          wkv_ = lambda k, o=wk_off: wkv_sb[
                              :, k * 256 + o: k * 256 + o + 128]

                          for step in range(KT + SKEW):
                              if step < KT:
                                  mm(a0, step, wq0, step == 0, step == KT - 1)
                              if step >= SKEW:
                                  k2 = step - SKEW
                                  mm(a1, k2, wq1, k2 == 0, k2 == KT - 1)
                                  mm(a2, k2, wkv_, k2 == 0, k2 == KT - 1)
                              # one interleaved action per step: drain the
                              # previous block's rope pops, and from block 2
                              # on feed qb0's attention between them.  After a
                              # head tail, hold off a few steps so its (slow)
                              # reciprocal drains before the next head's pd
                              # accumulation enters the in-order PE queue.
                              if step % 2 == 0 and pending:
                                  pending.pop(0)()
                              elif sb_i >= 2:
                                  if cooldown[0] > 0:
                                      cooldown[0] -= 1
                                  else:
                                      ev = gen_step()
                                      if ev is not None and ev[0] == "head":
                                          cooldown[0] = 5

                          # evacuate this pass; rope/transposes deferred
                          qsl0 = slice(m0 * SB, (m0 + 1) * SB)
                          qsl1 = slice(m1 * SB, (m1 + 1) * SB)
                          nc.scalar.copy(qTs[sb_i][:, qsl0], a0[:])
                          nc.vector.tensor_copy(qTs[sb_i][:, qsl1], a1[:])
                          newpend.append(lambda qsl=qsl0, b=sb_i, fn=fn, sl=sl:
                                         fn(qTs[b], qsl, sl))
                          newpend.append(lambda qsl=qsl1, b=sb_i, fn=fn, sl=sl:
                                         fn(qTs[b], qsl, sl))
                          if pas == 0:
                              nc.vector.tensor_copy(kTs[sb_i][:], a2[:])
                              newpend.append(lambda b=sb_i, fn=fn, sl=sl:
                                             fn(kTs[b], slice(0, SB), sl))
                          else:
                              vtmp = vt.tile([128, SB], BF16, tag="vtmp")
                              nc.scalar.copy(vtmp[:], a2[:])

                              def emit_transposes(sb_i=sb_i, vtmp=vtmp):
                                  # bf16 V transposes via the DMA XBAR: no
                                  # PE/DVE/PSUM involvement at all
                                  for tt in range(4):
                                      nc.sync.dma_start_transpose(
                                          out=vnats[sb_i][:, tt * 128:
                                                          (tt + 1) * 128],
                                          in_=vtmp[:, tt * 128:(tt + 1) * 128],
                                      )
                              newpend.append(emit_transposes)
                      for fnp in pending:   # anything not yet flushed
                          fnp()
                      pending = newpend

                  # qb0 leftovers that didn't fit into blocks 2-3
                  while not genq0_done[0]:
                      gen_step()

              # ============ stage 3+4: qb1..qb3 + output projection ==========
              with (
                  tc.tile_pool(name="psL", bufs=2, space="PSUM") as psL,
                  tc.tile_pool(name="psO", bufs=2, space="PSUM") as psO,
                  tc.tile_pool(name="psD", bufs=2, space="PSUM") as psD,
                  tc.tile_pool(name="psW", bufs=2, space="PSUM") as psW,
              ):
                  def stage4_chunk(qbx, tl, atiles, dve_copies=True):
                      # out[q-chunk t, :] partial = attn(:, t-cols) @ Wo
                      t = qbx * 4 + tl
                      obuf = ob.tile([128, D], BF16, tag="obuf")
                      for n4 in range(4):
                          pw = psW.tile([128, SB], F32, tag="pw")
                          for hh in range(HPC):
                              ah = atiles[hh]
                              lhs = (ah[tl][:] if isinstance(ah, list)
                                     else ah[:, tl * 128:(tl + 1) * 128])
                              nc.tensor.matmul(
                                  pw[:], lhs,
                                  wo_sb[:, hh * D + n4 * SB:
                                        hh * D + (n4 + 1) * SB],
                                  start=(hh == 0), stop=(hh == HPC - 1),
                              )
                          o0 = n4 * SB
                          # split PSUM→SBUF copies between ACT and DVE: ACT
                          # also carries every exp, DVE the normalization
                          if dve_copies and n4 % 2 == 0:
                              nc.vector.tensor_copy(obuf[:, o0:o0 + SB], pw[:])
                          else:
                              nc.scalar.copy(obuf[:, o0:o0 + SB], pw[:])
                          if n4 == 1:
                              nc.sync.dma_start(
                                  out=out[t * 128:(t + 1) * 128, 0:2 * SB],
                                  in_=obuf[:, 0:2 * SB],
                              )
                          elif n4 == 3:
                              nc.sync.dma_start(
                                  out=out[t * 128:(t + 1) * 128, 2 * SB:D],
                                  in_=obuf[:, 2 * SB:D],
                              )

                  prev_at = atiles0
                  prev_qb = 0
                  for qi, qb in enumerate([1, 2, 3]):
                      atiles = [at.tile([128, SB], BF16, tag=f"aT{h}",
                                        name=f"attnT_{qb}_h{h}")
                                for h in range(HPC)]
                      split3 = None
                      if qb == 3:
                          split3 = [at.tile([128, 128], BF16, tag=f"aT3s{t}",
                                            name=f"attnT_3_h3_{t}")
                                    for t in range(4)]
                          atiles[HPC - 1] = split3
                      for ev, h in make_attn_gen(qb, psL, psO, psD, atiles,
                                                 split3=split3):
                          if ev != "head":
                              continue
                          stage4_chunk(prev_qb, h, prev_at)
                          # drain block3's deferred ropes/transposes across
                          # qb1+qb2's heads (qb3 reads them)
                          if qi <= 1 and pending:
                              pending.pop(0)()
                      prev_at = atiles
                      prev_qb = qb
                  for tl in range(4):
                      stage4_chunk(prev_qb, tl, prev_at, dve_copies=False)
    _split_multiwaits(nc)
    return nc


_NC = None


def _get_nc():
    global _NC
    if _NC is None:
        _NC = build_nc()
    return _NC


def _make_in_maps(hidden, position_ids, Wqa, Wqb, Wk, Wv, Wo):
    hidden = np.asarray(hidden, dtype=np.float32)
    position_ids = np.asarray(position_ids)
    Wqa = np.asarray(Wqa, dtype=np.float32)
    Wqb = np.asarray(Wqb, dtype=np.float32)
    Wk = np.asarray(Wk, dtype=np.float32)
    Wv = np.asarray(Wv, dtype=np.float32)
    Wo = np.asarray(Wo, dtype=np.float32)
    weff_full = Wqa @ Wqb  # [D, H*HD]; exact assoc. fold of the LoRA Q proj

    inv_freq = 1.0 / (ROPE_BASE ** (np.arange(0, ROT, 2, dtype=np.float32) / ROT))
    in_maps = []
    for c in range(N_CORES):
        b, g = c // KVH, c % KVH
        pos = position_ids[b].astype(np.float32)
        freqs = pos[:, None] * inv_freq[None, :]        # [S, 32]
        cosT = np.cos(freqs).T.astype(np.float32)       # [32, S]
        sinT = np.sin(freqs).T.astype(np.float32)
        rcs = np.concatenate([cosT, cosT, sinT, sinT], axis=0).astype(
            ml_dtypes.bfloat16)  # [128, S]
        hsb = (hidden[b].T.reshape(KT, 128, NSB, SB)
               .transpose(1, 2, 0, 3).reshape(128, NSB * KT * SB))
        weff = (weff_full[:, g * HPC * HD:(g + 1) * HPC * HD]
                .reshape(KT, 128, 512).transpose(1, 0, 2).reshape(128, KT * 512))
        wkv = np.concatenate(
            [Wk[:, g * HD:(g + 1) * HD], Wv[:, g * HD:(g + 1) * HD]], axis=1
        ).reshape(KT, 128, 256).transpose(1, 0, 2).reshape(128, KT * 256)
        wog = (Wo[g * HPC * HD:(g + 1) * HPC * HD, :]
               .reshape(HPC, 128, D).transpose(1, 0, 2).reshape(128, HPC * D))
        in_maps.append({
            "hid": np.ascontiguousarray(hsb.astype(ml_dtypes.bfloat16)),
            "weff": np.ascontiguousarray(weff.astype(ml_dtypes.bfloat16)),
            "wkv": np.ascontiguousarray(wkv.astype(ml_dtypes.bfloat16)),
            "wo": np.ascontiguousarray(wog.astype(ml_dtypes.bfloat16)),
            "rcs": np.ascontiguousarray(rcs),
        })
    return in_maps


def _run(inputs, trace=False):
    nc = _get_nc()
    in_maps = _make_in_maps(**inputs)
    res = run_bass_kernel_spmd(nc, in_maps, list(range(N_CORES)), trace=trace)
    out = np.zeros((B, S, D), dtype=np.float32)
    for c in range(N_CORES):
        out[c // KVH] += res.results[c]["out"].astype(np.float32)
    return out, res


def kernel(**inputs) -> np.ndarray:
    return _run(inputs, trace=False)[0]
